# revision 1
# baseline (speedup 1.0000x reference)
"""GAT 3-layer (DiffusionOrderingNetwork) Trainium2 kernel, 8-core SPMD.

Strategy: nodes partitioned 8x2500 by dst; per-core ELL (degree-sorted,
per-tile width) edge layout; per-edge gathers via dma_gather 256B tokens
from an HBM node table [20480, 64] = [payload(36) | al_src(6) | al_dst(6) |
pad]; segment softmax uses a constant-shift exp (no segment max needed —
mathematically identical after normalization); messages multiplied+reduced
densely on DVE; AllGather rebuilds the table between layers.
"""

import sys

sys.path.insert(0, "/opt/trn_rl_repo")

import numpy as np
import concourse.bass as bass
import concourse.bacc as bacc
import concourse.mybir as mybir
import concourse.tile as tile
import concourse.bass_isa as bass_isa
from concourse import library_config
from concourse.bass_utils import run_bass_kernel_spmd

N = 20000
NC = 8
NPC = 2500          # nodes per core
NT = 20             # node tiles per core (128 rows each; 2560 padded rows)
NPP = NT * 128      # 2560 padded nodes per core
NTBL = NC * NPP     # 20480 table rows
H = 6
D_IN = 32
HC = 36             # heads * hidden
TW = 64             # table row width (f32) = 256B token
F32 = mybir.dt.float32
I16 = mybir.dt.int16


def _blockdiag(a):
    # a: [H, C] -> [H*C, H] with col h = a[h] at rows h*C:(h+1)*C
    Hh, C = a.shape
    out = np.zeros((Hh * C, Hh), np.float32)
    for h in range(Hh):
        out[h * C:(h + 1) * C, h] = a[h]
    return out


def _prep(x, edge_index, W1, a_src1, a_dst1, W2, a_src2, a_dst2, W3, a_src3, a_dst3):
    src = np.concatenate([edge_index[0], np.arange(N)]).astype(np.int64)
    dst = np.concatenate([edge_index[1], np.arange(N)]).astype(np.int64)

    deg = np.bincount(dst, minlength=N)
    # per-core degree-sorted node order; global_pos[n] = table row of node n
    orders = []          # per core: local pos -> global node id
    global_pos = np.zeros(N, np.int64)
    for k in range(NC):
        d = deg[k * NPC:(k + 1) * NPC]
        order = np.argsort(-d, kind="stable") + k * NPC
        orders.append(order)
        global_pos[order] = k * NPP + np.arange(NPC)

    # shared tile width schedule (max across cores)
    Wt = np.zeros(NT, np.int64)
    for k in range(NC):
        ds_ = np.sort(deg[k * NPC:(k + 1) * NPC])[::-1]
        ds_ = np.concatenate([ds_, np.zeros(NPP - NPC, np.int64)])
        Wt = np.maximum(Wt, ds_.reshape(NT, 128).max(axis=1))
    Wt = np.maximum((Wt + 3) // 4 * 4, 4).astype(np.int64)

    # CSR by dst
    sort_by_dst = np.argsort(dst, kind="stable")
    src_s = src[sort_by_dst]
    rowptr = np.zeros(N + 1, np.int64)
    np.cumsum(deg, out=rowptr[1:])

    idx_w = []   # per core: [128, 8*sum(Wt)] int16 wrapped index stream
    maskn = []   # per core: [128, sum(Wt)] f32 (0 valid, -1e4 pad)
    for k in range(NC):
        order = orders[k]
        iw_parts, mn_parts = [], []
        for t in range(NT):
            w = int(Wt[t])
            ell = np.zeros((128, w), np.int64)
            mn = np.full((128, w), -10000.0, np.float32)
            for p in range(128):
                li = t * 128 + p
                if li < NPC:
                    n = order[li]
                    e0, e1 = rowptr[n], rowptr[n + 1]
                    dd = int(e1 - e0)
                    ell[p, :dd] = global_pos[src_s[e0:e1]]
                    mn[p, :dd] = 0.0
                else:
                    mn[p, 0] = 0.0  # pad row: one live slot so denom > 0
            stream = ell.T.reshape(-1)            # slot-major: s*128+p
            iw = stream.reshape(-1, 16).T         # [16, 8w]
            iw_parts.append(np.tile(iw, (8, 1))[:128])  # replicate block to 128
            mn_parts.append(mn)
        idx_w.append(np.concatenate(iw_parts, axis=1).astype(np.int16))
        maskn.append(np.concatenate(mn_parts, axis=1))

    # x in permuted order, padded, transposed: [32, 20480]
    xp = np.zeros((NTBL, D_IN), np.float32)
    for k in range(NC):
        xp[k * NPP:k * NPP + NPC] = x[orders[k]]
    xT = np.ascontiguousarray(xp.T)

    Wc1 = np.concatenate([W1, W1 @ _blockdiag(a_src1), W1 @ _blockdiag(a_dst1)], 1)
    Wc2 = np.concatenate([W2, W2 @ _blockdiag(a_src2), W2 @ _blockdiag(a_dst2)], 1)
    I36 = np.eye(HC, dtype=np.float32)
    Wc3 = np.concatenate([I36, W3 @ _blockdiag(a_src3), W3 @ _blockdiag(a_dst3)], 1)

    unperm = np.concatenate(orders)  # row i of stacked core outputs -> node id
    return Wt, idx_w, maskn, xT, Wc1, Wc2, Wc3, unperm


def _build(nc, Wt):
    SWt = int(Wt.sum())
    CIDX = 8 * SWt

    # external inputs (per-core data passed via in_maps)
    t_idx = nc.dram_tensor("idxw", [128, CIDX], I16, kind="ExternalInput")
    t_mn = nc.dram_tensor("maskn", [128, SWt], F32, kind="ExternalInput")
    t_xT = nc.dram_tensor("xT", [D_IN, NTBL], F32, kind="ExternalInput")
    t_wc1 = nc.dram_tensor("wc1", [D_IN, 48], F32, kind="ExternalInput")
    t_wc2 = nc.dram_tensor("wc2", [HC, 48], F32, kind="ExternalInput")
    t_wc3 = nc.dram_tensor("wc3", [HC, 48], F32, kind="ExternalInput")
    t_w3 = nc.dram_tensor("w3", [HC, 192], F32, kind="ExternalInput")
    t_b1 = nc.dram_tensor("b1b", [128, HC], F32, kind="ExternalInput")
    t_b2 = nc.dram_tensor("b2b", [128, HC], F32, kind="ExternalInput")
    t_b3 = nc.dram_tensor("b3b", [128, D_IN], F32, kind="ExternalInput")
    t_rm = nc.dram_tensor("rowmask", [128, NT], F32, kind="ExternalInput")
    t_out = nc.dram_tensor("out", [NPP, D_IN], F32, kind="ExternalOutput")

    with tile.TileContext(nc) as tc:
        with (
            tc.tile_pool(name="dram", bufs=1, space="DRAM") as dram,
            tc.tile_pool(name="cst", bufs=1) as cst,
            tc.tile_pool(name="gat", bufs=4) as gat,
            tc.tile_pool(name="wrk", bufs=3) as wrk,
            tc.tile_pool(name="acc", bufs=1) as acc,
            tc.tile_pool(name="ps", bufs=2, space="PSUM") as ps,
        ):
            nc.gpsimd.load_library(library_config.mlp)
            TBL = dram.tile([NTBL, TW], F32)
            TBLS = [dram.tile([NTBL, TW], F32, addr_space="Shared", name="tbls2", tag="tbls2"),
                    dram.tile([NTBL, TW], F32, addr_space="Shared", name="tbls3", tag="tbls3")]
            BNC = dram.tile([NPP, TW], F32)
            CCI = dram.tile([32, 1], F32)
            CCO = dram.tile([32, 1], F32)

            sb_idx = cst.tile([128, CIDX], I16)
            nc.sync.dma_start(out=sb_idx[:], in_=t_idx[:])
            sb_mn = cst.tile([128, SWt], F32)
            nc.sync.dma_start(out=sb_mn[:], in_=t_mn[:])
            sb_wc = [cst.tile([D_IN, 48], F32, tag="wc0", name="wc0"),
                     cst.tile([HC, 48], F32, tag="wc1t", name="wc1t"),
                     cst.tile([HC, 48], F32, tag="wc2t", name="wc2t")]
            nc.sync.dma_start(out=sb_wc[0][:], in_=t_wc1[:])
            nc.sync.dma_start(out=sb_wc[1][:], in_=t_wc2[:])
            nc.sync.dma_start(out=sb_wc[2][:], in_=t_wc3[:])
            sb_w3 = cst.tile([HC, 192], F32)
            nc.sync.dma_start(out=sb_w3[:], in_=t_w3[:])
            sb_b = [cst.tile([128, HC], F32, tag="b0", name="b0"),
                    cst.tile([128, HC], F32, tag="b1t", name="b1t"),
                    cst.tile([128, D_IN], F32, tag="b2t", name="b2t")]
            nc.sync.dma_start(out=sb_b[0][:], in_=t_b1[:])
            nc.sync.dma_start(out=sb_b[1][:], in_=t_b2[:])
            nc.sync.dma_start(out=sb_b[2][:], in_=t_b3[:])
            t_id = nc.dram_tensor("ident", [128, 128], F32, kind="ExternalInput")
            ident = cst.tile([128, 128], F32)
            nc.sync.dma_start(out=ident[:], in_=t_id[:])
            bm20 = cst.tile([128, 1], F32)
            nc.vector.memset(bm20[:], -20.0)
            bm50 = cst.tile([128, 1], F32)
            nc.vector.memset(bm50[:], -50.0)

            # persistent per-layer state
            ald_own = [acc.tile([128, NT, H], F32, tag="ald0", name="ald0"),
                       acc.tile([128, NT, H], F32, tag="ald1", name="ald1")]
            h_all = acc.tile([128, NT, HC], F32)
            e3_all = acc.tile([128, NT, D_IN], F32)

            # ---- layer-1 table: TBL = x_perm @ Wc1, all rows ----
            for b in range(NTBL // 128):
                xblk = wrk.tile([D_IN, 128], F32, tag="xblk")
                nc.sync.dma_start(out=xblk[:], in_=t_xT[:, b * 128:(b + 1) * 128])
                pt = ps.tile([128, 48], F32, tag="tb")
                nc.tensor.matmul(pt[:], xblk[:], sb_wc[0][:],
                                 start=True, stop=True)
                sb_tb = wrk.tile([128, 48], F32, tag="tbs")
                nc.scalar.activation(sb_tb[:], pt[:],
                                     mybir.ActivationFunctionType.Copy)
                nc.sync.dma_start(out=TBL[b * 128:(b + 1) * 128, 0:48], in_=sb_tb[:])

            cur = 0  # ald_own buffer index; filled from own-block cols
            ko = None  # own rows = input xT_own? use global: own block is rank-dep
            # ald for layer 1: own rows ald = (x_own @ Wc1)[:, 42:48].
            # Own x rows are provided via xT_own input to stay SPMD-uniform.
            t_xTo = nc.dram_tensor("xT_own", [D_IN, NPP], F32, kind="ExternalInput")
            sb_xTo = cst.tile([D_IN, NPP], F32)
            nc.sync.dma_start(out=sb_xTo[:], in_=t_xTo[:])
            for t in range(NT):
                pt = ps.tile([128, 48], F32, tag="tb")
                nc.tensor.matmul(pt[:], sb_xTo[:, t * 128:(t + 1) * 128], sb_wc[0][:],
                                 start=True, stop=True)
                nc.scalar.activation(ald_own[0][:, t, :], pt[:, 42:48],
                                     mybir.ActivationFunctionType.Copy)

            # ---- layers ----
            qctr = 0
            for li in range(3):
                ioff = 0
                moff = 0
                for t in range(NT):
                    w = int(Wt[t])
                    ni = 128 * w
                    G = gat.tile([128, w, TW], F32, tag="G")
                    for c in range(0, w, 8):
                        cw = min(8, w - c)
                        cni = 128 * cw
                        nc.gpsimd.dma_gather(
                            out_ap=G[:, c:c + cw, :],
                            in_ap=(TBL if li == 0 else TBLS[li - 1])[:],
                            idxs_ap=sb_idx[:, ioff + 8 * c:ioff + 8 * (c + cw)],
                            num_idxs=cni, num_idxs_reg=cni, elem_size=TW,
                            queue_num=qctr % 4,
                        )
                        qctr += 1
                    # aldm[p,s,h] = ald_own[p,h] + maskneg[p,s]
                    aldm = wrk.tile([128, w, H], F32, tag="aldm")
                    nc.vector.tensor_tensor(
                        out=aldm[:],
                        in0=ald_own[li % 2][:, t, :][:, None, :].broadcast_to([128, w, H]),
                        in1=sb_mn[:, moff:moff + w][:, :, None].broadcast_to([128, w, H]),
                        op=mybir.AluOpType.add)
                    lg = wrk.tile([128, w, H], F32, tag="lg")
                    nc.vector.tensor_tensor(out=lg[:], in0=G[:, :, HC:HC + H],
                                            in1=aldm[:], op=mybir.AluOpType.add)
                    lgs = wrk.tile([128, w, H], F32, tag="lgs")
                    nc.vector.tensor_scalar_mul(lgs[:], lg[:], 0.2)
                    nc.vector.tensor_max(lg[:], lg[:], lgs[:])
                    ex = wrk.tile([128, w, H], F32, tag="ex")
                    nc.scalar.activation(ex[:], lg[:],
                                         mybir.ActivationFunctionType.Exp,
                                         bias=bm20[:])
                    den = wrk.tile([128, H], F32, tag="den")
                    nc.vector.tensor_reduce(
                        out=den[:], in_=ex[:].rearrange("p s h -> p h s"),
                        axis=mybir.AxisListType.X, op=mybir.AluOpType.add)
                    rd = wrk.tile([128, H], F32, tag="rd")
                    nc.vector.reciprocal(rd[:], den[:])
                    if li < 2:
                        msg = wrk.tile([128, w, H, H], F32, tag="msg")
                        nc.vector.tensor_tensor(
                            out=msg[:],
                            in0=ex[:][:, :, :, None].broadcast_to([128, w, H, H]),
                            in1=G[:, :, 0:HC].rearrange("p s (h c) -> p s h c", h=H),
                            op=mybir.AluOpType.mult)
                        agg = wrk.tile([128, HC], F32, tag="agg")
                        nc.vector.tensor_reduce(
                            out=agg[:], in_=msg[:].rearrange("p s h c -> p (h c) s"),
                            axis=mybir.AxisListType.X, op=mybir.AluOpType.add)
                        hp = wrk.tile([128, HC], F32, tag="hp")
                        nc.vector.tensor_tensor(
                            out=hp[:].rearrange("p (h c) -> p h c", h=H),
                            in0=agg[:].rearrange("p (h c) -> p h c", h=H),
                            in1=rd[:][:, :, None].broadcast_to([128, H, H]),
                            op=mybir.AluOpType.mult)
                        nc.vector.tensor_tensor(out=hp[:], in0=hp[:], in1=sb_b[li][:],
                                                op=mybir.AluOpType.add)
                        nc.scalar.activation(h_all[:, t, :], hp[:],
                                             mybir.ActivationFunctionType.Relu)
                    else:
                        agg3 = wrk.tile([128, H, HC], F32, tag="agg3")
                        for h in range(H):
                            m3 = wrk.tile([128, w, HC], F32, tag="m3")
                            nc.vector.tensor_tensor(
                                out=m3[:],
                                in0=ex[:, :, h][:, :, None].broadcast_to([128, w, HC]),
                                in1=G[:, :, 0:HC],
                                op=mybir.AluOpType.mult)
                            nc.vector.tensor_reduce(
                                out=agg3[:, h, :],
                                in_=m3[:].rearrange("p s c -> p c s"),
                                axis=mybir.AxisListType.X, op=mybir.AluOpType.add)
                        nc.vector.tensor_tensor(
                            out=agg3[:],
                            in0=agg3[:],
                            in1=rd[:][:, :, None].broadcast_to([128, H, HC]),
                            op=mybir.AluOpType.mult)
                        zp = ps.tile([128, D_IN], F32, tag="z")
                        for h in range(H):
                            tp = ps.tile([36, 128], F32, tag="tp")
                            nc.tensor.transpose(tp[:], agg3[:, h, :], ident[:])
                            ts = wrk.tile([36, 128], F32, tag="ts")
                            nc.scalar.activation(ts[:], tp[:],
                                                 mybir.ActivationFunctionType.Copy)
                            nc.tensor.matmul(zp[:], ts[:],
                                             sb_w3[:, h * 32:(h + 1) * 32],
                                             start=(h == 0), stop=(h == 5))
                        zs = wrk.tile([128, D_IN], F32, tag="zs")
                        nc.vector.tensor_scalar_mul(zs[:], zp[:], 1.0 / 6.0)
                        nc.vector.tensor_tensor(out=zs[:], in0=zs[:], in1=sb_b[2][:],
                                                op=mybir.AluOpType.add)
                        nc.scalar.activation(e3_all[:, t, :], zs[:],
                                             mybir.ActivationFunctionType.Exp,
                                             bias=bm50[:])
                    ioff += 8 * w
                    moff += w

                if li < 2:
                    # table build for next layer + AllGather
                    for t in range(NT):
                        tp = ps.tile([36, 128], F32, tag="tp")
                        nc.tensor.transpose(tp[:], h_all[:, t, :], ident[:])
                        ts = wrk.tile([36, 128], F32, tag="ts")
                        nc.scalar.activation(ts[:], tp[:],
                                             mybir.ActivationFunctionType.Copy)
                        pt = ps.tile([128, 48], F32, tag="tb")
                        nc.tensor.matmul(pt[:], ts[:], sb_wc[li + 1][:],
                                         start=True, stop=True)
                        nc.scalar.activation(ald_own[(li + 1) % 2][:, t, :],
                                             pt[:, 42:48],
                                             mybir.ActivationFunctionType.Copy)
                        sb_tb = wrk.tile([128, 48], F32, tag="tbs")
                        nc.scalar.activation(sb_tb[:], pt[:],
                                             mybir.ActivationFunctionType.Copy)
                        nc.sync.dma_start(out=BNC[t * 128:(t + 1) * 128, 0:48],
                                          in_=sb_tb[:])
                    tc.strict_bb_all_engine_barrier()
                    nc.gpsimd.collective_compute(
                        "AllGather", mybir.AluOpType.bypass,
                        replica_groups=[list(range(NC))],
                        ins=[BNC[:].opt()], outs=[TBLS[li][:].opt()])
                    tc.strict_bb_all_engine_barrier()

            # ---- global softmax over nodes ----
            sb_rm = cst.tile([128, NT], F32)
            nc.sync.dma_start(out=sb_rm[:], in_=t_rm[:])
            nc.vector.tensor_tensor(
                out=e3_all[:], in0=e3_all[:],
                in1=sb_rm[:][:, :, None].broadcast_to([128, NT, D_IN]),
                op=mybir.AluOpType.mult)
            s0 = wrk.tile([128, D_IN], F32, tag="s0")
            nc.vector.tensor_reduce(out=s0[:],
                                    in_=e3_all[:].rearrange("p t c -> p c t"),
                                    axis=mybir.AxisListType.X, op=mybir.AluOpType.add)
            tp2 = ps.tile([32, 128], F32, tag="tp")
            nc.tensor.transpose(tp2[:], s0[:], ident[:])
            ts2 = wrk.tile([32, 128], F32, tag="ts2")
            nc.scalar.activation(ts2[:], tp2[:],
                                 mybir.ActivationFunctionType.Copy)
            red = wrk.tile([32, 1], F32, tag="red")
            nc.vector.tensor_reduce(out=red[:], in_=ts2[:],
                                    axis=mybir.AxisListType.X,
                                    op=mybir.AluOpType.add)
            nc.sync.dma_start(out=CCI[:], in_=red[:])
            tc.strict_bb_all_engine_barrier()
            nc.gpsimd.collective_compute(
                "AllReduce", mybir.AluOpType.add,
                replica_groups=[list(range(NC))],
                ins=[CCI[:].opt()], outs=[CCO[:].opt()])
            tc.strict_bb_all_engine_barrier()
            ssum = wrk.tile([32, 1], F32, tag="ssum")
            nc.sync.dma_start(out=ssum[:], in_=CCO[:])
            rc32 = wrk.tile([32, 1], F32, tag="rc32")
            nc.vector.reciprocal(rc32[:], ssum[:])
            rp1 = ps.tile([1, 32], F32, tag="tp")
            nc.tensor.transpose(rp1[:], rc32[:], ident[0:32, 0:32])
            rs1 = wrk.tile([1, 32], F32, tag="rs1")
            nc.scalar.activation(rs1[:], rp1[:],
                                 mybir.ActivationFunctionType.Copy)
            ones = cst.tile([1, 128], F32)
            nc.vector.memset(ones[:], 1.0)
            rbp = ps.tile([128, D_IN], F32, tag="z")
            nc.tensor.matmul(rbp[:], ones[:], rs1[:], start=True, stop=True)
            rb = wrk.tile([128, D_IN], F32, tag="rb")
            nc.scalar.activation(rb[:], rbp[:],
                                 mybir.ActivationFunctionType.Copy)
            ot = wrk.tile([128, NT, D_IN], F32, tag="ot")
            nc.vector.tensor_tensor(
                out=ot[:], in0=e3_all[:],
                in1=rb[:][:, None, :].broadcast_to([128, NT, D_IN]),
                op=mybir.AluOpType.mult)
            nc.sync.dma_start(
                out=t_out[:].rearrange("(t p) c -> p t c", p=128), in_=ot[:])
    return nc


_CACHE = {}
LAST_EXEC_NS = None


def kernel(x, edge_index, W1, a_src1, a_dst1, b1, W2, a_src2, a_dst2, b2,
           W3, a_src3, a_dst3, b3):
    x = np.asarray(x, np.float32)
    edge_index = np.asarray(edge_index)
    args = [np.asarray(a, np.float32) for a in
            (W1, a_src1, a_dst1, W2, a_src2, a_dst2, W3, a_src3, a_dst3)]
    Wt, idx_w, maskn, xT, Wc1, Wc2, Wc3, unperm = _prep(x, edge_index, *args)

    nc = bacc.Bacc(None, num_devices=NC, num_swdge_queues=4)
    nc = _build(nc, Wt)
    nc.compile()

    rowmask = (np.arange(NPP).reshape(NT, 128).T < NPC).astype(np.float32)
    rowmask = (np.arange(NT)[None, :] * 128 + np.arange(128)[:, None] < NPC).astype(np.float32)
    in_maps = []
    for k in range(NC):
        in_maps.append({
            "idxw": idx_w[k],
            "maskn": maskn[k],
            "xT": xT,
            "xT_own": np.ascontiguousarray(xT[:, k * NPP:(k + 1) * NPP]),
            "wc1": Wc1, "wc2": Wc2, "wc3": Wc3,
            "w3": args[6],
            "b1b": np.broadcast_to(np.asarray(b1, np.float32), (128, HC)).copy(),
            "b2b": np.broadcast_to(np.asarray(b2, np.float32), (128, HC)).copy(),
            "b3b": np.broadcast_to(np.asarray(b3, np.float32), (128, D_IN)).copy(),
            "ident": np.eye(128, dtype=np.float32),
            "rowmask": rowmask,
        })
    import time as _time
    _t0 = _time.time()
    res = run_bass_kernel_spmd(nc, in_maps, core_ids=list(range(NC)))
    global LAST_EXEC_NS
    LAST_EXEC_NS = res.exec_time_ns or int((_time.time() - _t0) * 1e9)
    outs = [res.results[k]["out"] for k in range(NC)]
    stacked = np.concatenate([o[:NPC] for o in outs], axis=0)
    full = np.zeros((N, D_IN), np.float32)
    full[unperm] = stacked
    return full



# revision 3
# speedup vs baseline: 1.2417x; 1.2417x over previous
"""GAT 3-layer (DiffusionOrderingNetwork) Trainium2 kernel, 8-core SPMD.

Strategy: nodes partitioned 8x2500 by dst; per-core ELL (degree-sorted,
per-tile width) edge layout; per-edge gathers via dma_gather 256B tokens
from an HBM node table [20480, 64] = [payload(36) | al_src(6) | al_dst(6) |
pad]; segment softmax uses a constant-shift exp (no segment max needed —
mathematically identical after normalization); messages multiplied+reduced
densely on DVE; AllGather rebuilds the table between layers.
"""

import sys

sys.path.insert(0, "/opt/trn_rl_repo")

import numpy as np
import concourse.bass as bass
import concourse.bacc as bacc
import concourse.mybir as mybir
import concourse.tile as tile
import concourse.bass_isa as bass_isa
from concourse import library_config
from concourse.bass_utils import run_bass_kernel_spmd

N = 20000
NC = 8
NPC = 2500          # nodes per core
NT = 20             # node tiles per core (128 rows each; 2560 padded rows)
NPP = NT * 128      # 2560 padded nodes per core
NTBL = NC * NPP     # 20480 table rows
H = 6
D_IN = 32
HC = 36             # heads * hidden
TW = 64             # table row width (f32) = 256B token
F32 = mybir.dt.float32
I16 = mybir.dt.int16


def _blockdiag(a):
    # a: [H, C] -> [H*C, H] with col h = a[h] at rows h*C:(h+1)*C
    Hh, C = a.shape
    out = np.zeros((Hh * C, Hh), np.float32)
    for h in range(Hh):
        out[h * C:(h + 1) * C, h] = a[h]
    return out


def _prep(x, edge_index, W1, a_src1, a_dst1, W2, a_src2, a_dst2, W3, a_src3, a_dst3):
    src = np.concatenate([edge_index[0], np.arange(N)]).astype(np.int64)
    dst = np.concatenate([edge_index[1], np.arange(N)]).astype(np.int64)

    deg = np.bincount(dst, minlength=N)
    # per-core degree-sorted node order; global_pos[n] = table row of node n
    orders = []          # per core: local pos -> global node id
    global_pos = np.zeros(N, np.int64)
    for k in range(NC):
        d = deg[k * NPC:(k + 1) * NPC]
        order = np.argsort(-d, kind="stable") + k * NPC
        orders.append(order)
        global_pos[order] = k * NPP + np.arange(NPC)

    # shared tile width schedule (max across cores)
    Wt = np.zeros(NT, np.int64)
    for k in range(NC):
        ds_ = np.sort(deg[k * NPC:(k + 1) * NPC])[::-1]
        ds_ = np.concatenate([ds_, np.zeros(NPP - NPC, np.int64)])
        Wt = np.maximum(Wt, ds_.reshape(NT, 128).max(axis=1))
    Wt = np.maximum((Wt + 3) // 4 * 4, 4).astype(np.int64)

    # CSR by dst
    sort_by_dst = np.argsort(dst, kind="stable")
    src_s = src[sort_by_dst]
    rowptr = np.zeros(N + 1, np.int64)
    np.cumsum(deg, out=rowptr[1:])

    idx_w = []   # per core: [128, 8*sum(Wt)] int16 wrapped index stream
    maskn = []   # per core: [128, sum(Wt)] f32 (0 valid, -1e4 pad)
    for k in range(NC):
        order = orders[k]
        iw_parts, mn_parts = [], []
        for t in range(NT):
            w = int(Wt[t])
            ell = np.zeros((128, w), np.int64)
            mn = np.full((128, w), -10000.0, np.float32)
            for p in range(128):
                li = t * 128 + p
                if li < NPC:
                    n = order[li]
                    e0, e1 = rowptr[n], rowptr[n + 1]
                    dd = int(e1 - e0)
                    ell[p, :dd] = global_pos[src_s[e0:e1]]
                    mn[p, :dd] = 0.0
                else:
                    mn[p, 0] = 0.0  # pad row: one live slot so denom > 0
            stream = ell.T.reshape(-1)            # slot-major: s*128+p
            iw = stream.reshape(-1, 16).T         # [16, 8w]
            iw_parts.append(np.tile(iw, (8, 1))[:128])  # replicate block to 128
            mn_parts.append(mn)
        idx_w.append(np.concatenate(iw_parts, axis=1).astype(np.int16))
        maskn.append(np.concatenate(mn_parts, axis=1))

    # x in permuted order, padded, transposed: [32, 20480]
    xp = np.zeros((NTBL, D_IN), np.float32)
    for k in range(NC):
        xp[k * NPP:k * NPP + NPC] = x[orders[k]]
    xT = np.ascontiguousarray(xp.T)

    Wc1 = np.concatenate([W1, W1 @ _blockdiag(a_src1), W1 @ _blockdiag(a_dst1)], 1)
    Wc2 = np.concatenate([W2, W2 @ _blockdiag(a_src2), W2 @ _blockdiag(a_dst2)], 1)
    I36 = np.eye(HC, dtype=np.float32)
    Wc3 = np.concatenate([I36, W3 @ _blockdiag(a_src3), W3 @ _blockdiag(a_dst3)], 1)

    unperm = np.concatenate(orders)  # row i of stacked core outputs -> node id
    return Wt, idx_w, maskn, xT, Wc1, Wc2, Wc3, unperm


def _build(nc, Wt):
    SWt = int(Wt.sum())
    CIDX = 8 * SWt

    # external inputs (per-core data passed via in_maps)
    t_idx = nc.dram_tensor("idxw", [128, CIDX], I16, kind="ExternalInput")
    t_mn = nc.dram_tensor("maskn", [128, SWt], F32, kind="ExternalInput")
    t_xT = nc.dram_tensor("xT", [D_IN, NTBL], F32, kind="ExternalInput")
    t_wc1 = nc.dram_tensor("wc1", [D_IN, 48], F32, kind="ExternalInput")
    t_wc2 = nc.dram_tensor("wc2", [HC, 48], F32, kind="ExternalInput")
    t_wc3 = nc.dram_tensor("wc3", [HC, 48], F32, kind="ExternalInput")
    t_w3 = nc.dram_tensor("w3", [HC, 192], F32, kind="ExternalInput")
    t_b1 = nc.dram_tensor("b1b", [128, HC], F32, kind="ExternalInput")
    t_b2 = nc.dram_tensor("b2b", [128, HC], F32, kind="ExternalInput")
    t_b3 = nc.dram_tensor("b3b", [128, D_IN], F32, kind="ExternalInput")
    t_rm = nc.dram_tensor("rowmask", [128, NT], F32, kind="ExternalInput")
    t_out = nc.dram_tensor("out", [NPP, D_IN], F32, kind="ExternalOutput")

    with tile.TileContext(nc) as tc:
        with (
            tc.tile_pool(name="dram", bufs=1, space="DRAM") as dram,
            tc.tile_pool(name="cst", bufs=1) as cst,
            tc.tile_pool(name="gat", bufs=4) as gat,
            tc.tile_pool(name="wrk", bufs=3) as wrk,
            tc.tile_pool(name="acc", bufs=1) as acc,
            tc.tile_pool(name="ps", bufs=2, space="PSUM") as ps,
        ):
            nc.gpsimd.load_library(library_config.mlp)
            TBL = dram.tile([NTBL, TW], F32)
            TBLS = [dram.tile([NTBL, TW], F32, addr_space="Shared", name="tbls2", tag="tbls2"),
                    dram.tile([NTBL, TW], F32, addr_space="Shared", name="tbls3", tag="tbls3")]
            BNC = dram.tile([NPP, TW], F32)
            CCI = dram.tile([32, 1], F32)
            CCO = dram.tile([32, 1], F32)

            sb_idx = cst.tile([128, CIDX], I16)
            nc.sync.dma_start(out=sb_idx[:], in_=t_idx[:])
            sb_mn = cst.tile([128, SWt], F32)
            nc.sync.dma_start(out=sb_mn[:], in_=t_mn[:])
            sb_wc = [cst.tile([D_IN, 48], F32, tag="wc0", name="wc0"),
                     cst.tile([HC, 48], F32, tag="wc1t", name="wc1t"),
                     cst.tile([HC, 48], F32, tag="wc2t", name="wc2t")]
            nc.sync.dma_start(out=sb_wc[0][:], in_=t_wc1[:])
            nc.sync.dma_start(out=sb_wc[1][:], in_=t_wc2[:])
            nc.sync.dma_start(out=sb_wc[2][:], in_=t_wc3[:])
            sb_w3 = cst.tile([HC, 192], F32)
            nc.sync.dma_start(out=sb_w3[:], in_=t_w3[:])
            sb_b = [cst.tile([128, HC], F32, tag="b0", name="b0"),
                    cst.tile([128, HC], F32, tag="b1t", name="b1t"),
                    cst.tile([128, D_IN], F32, tag="b2t", name="b2t")]
            nc.sync.dma_start(out=sb_b[0][:], in_=t_b1[:])
            nc.sync.dma_start(out=sb_b[1][:], in_=t_b2[:])
            nc.sync.dma_start(out=sb_b[2][:], in_=t_b3[:])
            t_id = nc.dram_tensor("ident", [128, 128], F32, kind="ExternalInput")
            ident = cst.tile([128, 128], F32)
            nc.sync.dma_start(out=ident[:], in_=t_id[:])
            bm20 = cst.tile([128, 1], F32)
            nc.vector.memset(bm20[:], -20.0)
            bm50 = cst.tile([128, 1], F32)
            nc.vector.memset(bm50[:], -50.0)

            # persistent per-layer state
            ald_own = [acc.tile([128, NT, H], F32, tag="ald0", name="ald0"),
                       acc.tile([128, NT, H], F32, tag="ald1", name="ald1")]
            h_all = acc.tile([128, NT, HC], F32)
            e3_all = acc.tile([128, NT, D_IN], F32)

            # ---- layer-1 table: TBL = x_perm @ Wc1, all rows ----
            for b in range(NTBL // 128):
                xblk = wrk.tile([D_IN, 128], F32, tag="xblk")
                nc.sync.dma_start(out=xblk[:], in_=t_xT[:, b * 128:(b + 1) * 128])
                pt = ps.tile([128, 48], F32, tag="tb")
                nc.tensor.matmul(pt[:], xblk[:], sb_wc[0][:],
                                 start=True, stop=True)
                sb_tb = wrk.tile([128, 48], F32, tag="tbs")
                nc.scalar.activation(sb_tb[:], pt[:],
                                     mybir.ActivationFunctionType.Copy)
                nc.sync.dma_start(out=TBL[b * 128:(b + 1) * 128, 0:48], in_=sb_tb[:])

            cur = 0  # ald_own buffer index; filled from own-block cols
            ko = None  # own rows = input xT_own? use global: own block is rank-dep
            # ald for layer 1: own rows ald = (x_own @ Wc1)[:, 42:48].
            # Own x rows are provided via xT_own input to stay SPMD-uniform.
            t_xTo = nc.dram_tensor("xT_own", [D_IN, NPP], F32, kind="ExternalInput")
            sb_xTo = cst.tile([D_IN, NPP], F32)
            nc.sync.dma_start(out=sb_xTo[:], in_=t_xTo[:])
            for t in range(NT):
                pt = ps.tile([128, 48], F32, tag="tb")
                nc.tensor.matmul(pt[:], sb_xTo[:, t * 128:(t + 1) * 128], sb_wc[0][:],
                                 start=True, stop=True)
                nc.scalar.activation(ald_own[0][:, t, :], pt[:, 42:48],
                                     mybir.ActivationFunctionType.Copy)

            # ---- layers ----
            qctr = 0
            for li in range(3):
                ioff = 0
                moff = 0
                for t in range(NT):
                    w = int(Wt[t])
                    ni = 128 * w
                    G = gat.tile([128, w, TW], F32, tag="G")
                    for c in range(0, w, 8):
                        cw = min(8, w - c)
                        cni = 128 * cw
                        nc.gpsimd.dma_gather(
                            out_ap=G[:, c:c + cw, :],
                            in_ap=(TBL if li == 0 else TBLS[li - 1])[:],
                            idxs_ap=sb_idx[:, ioff + 8 * c:ioff + 8 * (c + cw)],
                            num_idxs=cni, num_idxs_reg=cni, elem_size=TW,
                            queue_num=qctr % 4,
                        )
                        qctr += 1
                    # aldm[p,s,h] = ald_own[p,h] + maskneg[p,s]
                    aldm = wrk.tile([128, w, H], F32, tag="aldm")
                    nc.vector.tensor_tensor(
                        out=aldm[:],
                        in0=ald_own[li % 2][:, t, :][:, None, :].broadcast_to([128, w, H]),
                        in1=sb_mn[:, moff:moff + w][:, :, None].broadcast_to([128, w, H]),
                        op=mybir.AluOpType.add)
                    lg = wrk.tile([128, w, H], F32, tag="lg")
                    nc.vector.tensor_tensor(out=lg[:], in0=G[:, :, HC:HC + H],
                                            in1=aldm[:], op=mybir.AluOpType.add)
                    lgs = wrk.tile([128, w, H], F32, tag="lgs")
                    nc.vector.tensor_scalar_mul(lgs[:], lg[:], 0.2)
                    nc.vector.tensor_max(lg[:], lg[:], lgs[:])
                    ex = wrk.tile([128, w, H], F32, tag="ex")
                    nc.scalar.activation(ex[:], lg[:],
                                         mybir.ActivationFunctionType.Exp,
                                         bias=bm20[:])
                    den = wrk.tile([128, H], F32, tag="den")
                    nc.vector.tensor_reduce(
                        out=den[:], in_=ex[:].rearrange("p s h -> p h s"),
                        axis=mybir.AxisListType.X, op=mybir.AluOpType.add)
                    rd = wrk.tile([128, H], F32, tag="rd")
                    nc.vector.reciprocal(rd[:], den[:])
                    if li < 2:
                        msg = wrk.tile([128, w, H, H], F32, tag="msg")
                        nc.vector.tensor_tensor(
                            out=msg[:],
                            in0=ex[:][:, :, :, None].broadcast_to([128, w, H, H]),
                            in1=G[:, :, 0:HC].rearrange("p s (h c) -> p s h c", h=H),
                            op=mybir.AluOpType.mult)
                        agg = wrk.tile([128, HC], F32, tag="agg")
                        nc.vector.tensor_reduce(
                            out=agg[:], in_=msg[:].rearrange("p s h c -> p (h c) s"),
                            axis=mybir.AxisListType.X, op=mybir.AluOpType.add)
                        hp = wrk.tile([128, HC], F32, tag="hp")
                        nc.vector.tensor_tensor(
                            out=hp[:].rearrange("p (h c) -> p h c", h=H),
                            in0=agg[:].rearrange("p (h c) -> p h c", h=H),
                            in1=rd[:][:, :, None].broadcast_to([128, H, H]),
                            op=mybir.AluOpType.mult)
                        nc.vector.tensor_tensor(out=hp[:], in0=hp[:], in1=sb_b[li][:],
                                                op=mybir.AluOpType.add)
                        nc.scalar.activation(h_all[:, t, :], hp[:],
                                             mybir.ActivationFunctionType.Relu)
                    else:
                        agg3 = wrk.tile([128, H, HC], F32, tag="agg3")
                        for h in range(H):
                            m3 = wrk.tile([128, w, HC], F32, tag="m3")
                            nc.vector.tensor_tensor(
                                out=m3[:],
                                in0=ex[:, :, h][:, :, None].broadcast_to([128, w, HC]),
                                in1=G[:, :, 0:HC],
                                op=mybir.AluOpType.mult)
                            nc.vector.tensor_reduce(
                                out=agg3[:, h, :],
                                in_=m3[:].rearrange("p s c -> p c s"),
                                axis=mybir.AxisListType.X, op=mybir.AluOpType.add)
                        nc.vector.tensor_tensor(
                            out=agg3[:],
                            in0=agg3[:],
                            in1=rd[:][:, :, None].broadcast_to([128, H, HC]),
                            op=mybir.AluOpType.mult)
                        zp = ps.tile([128, D_IN], F32, tag="z")
                        for h in range(H):
                            tp = ps.tile([36, 128], F32, tag="tp")
                            nc.tensor.transpose(tp[:], agg3[:, h, :], ident[:])
                            ts = wrk.tile([36, 128], F32, tag="ts")
                            nc.scalar.activation(ts[:], tp[:],
                                                 mybir.ActivationFunctionType.Copy)
                            nc.tensor.matmul(zp[:], ts[:],
                                             sb_w3[:, h * 32:(h + 1) * 32],
                                             start=(h == 0), stop=(h == 5))
                        zs = wrk.tile([128, D_IN], F32, tag="zs")
                        nc.vector.tensor_scalar_mul(zs[:], zp[:], 1.0 / 6.0)
                        nc.vector.tensor_tensor(out=zs[:], in0=zs[:], in1=sb_b[2][:],
                                                op=mybir.AluOpType.add)
                        nc.scalar.activation(e3_all[:, t, :], zs[:],
                                             mybir.ActivationFunctionType.Exp,
                                             bias=bm50[:])
                    ioff += 8 * w
                    moff += w

                if li < 2:
                    # table build for next layer + AllGather
                    for t in range(NT):
                        tp = ps.tile([36, 128], F32, tag="tp")
                        nc.tensor.transpose(tp[:], h_all[:, t, :], ident[:])
                        ts = wrk.tile([36, 128], F32, tag="ts")
                        nc.scalar.activation(ts[:], tp[:],
                                             mybir.ActivationFunctionType.Copy)
                        pt = ps.tile([128, 48], F32, tag="tb")
                        nc.tensor.matmul(pt[:], ts[:], sb_wc[li + 1][:],
                                         start=True, stop=True)
                        nc.scalar.activation(ald_own[(li + 1) % 2][:, t, :],
                                             pt[:, 42:48],
                                             mybir.ActivationFunctionType.Copy)
                        sb_tb = wrk.tile([128, 48], F32, tag="tbs")
                        nc.scalar.activation(sb_tb[:], pt[:],
                                             mybir.ActivationFunctionType.Copy)
                        nc.sync.dma_start(out=BNC[t * 128:(t + 1) * 128, 0:48],
                                          in_=sb_tb[:])
                    tc.strict_bb_all_engine_barrier()
                    nc.gpsimd.collective_compute(
                        "AllGather", mybir.AluOpType.bypass,
                        replica_groups=[list(range(NC))],
                        ins=[BNC[:].opt()], outs=[TBLS[li][:].opt()])
                    tc.strict_bb_all_engine_barrier()

            # ---- global softmax over nodes ----
            sb_rm = cst.tile([128, NT], F32)
            nc.sync.dma_start(out=sb_rm[:], in_=t_rm[:])
            nc.vector.tensor_tensor(
                out=e3_all[:], in0=e3_all[:],
                in1=sb_rm[:][:, :, None].broadcast_to([128, NT, D_IN]),
                op=mybir.AluOpType.mult)
            s0 = wrk.tile([128, D_IN], F32, tag="s0")
            nc.vector.tensor_reduce(out=s0[:],
                                    in_=e3_all[:].rearrange("p t c -> p c t"),
                                    axis=mybir.AxisListType.X, op=mybir.AluOpType.add)
            tp2 = ps.tile([32, 128], F32, tag="tp")
            nc.tensor.transpose(tp2[:], s0[:], ident[:])
            ts2 = wrk.tile([32, 128], F32, tag="ts2")
            nc.scalar.activation(ts2[:], tp2[:],
                                 mybir.ActivationFunctionType.Copy)
            red = wrk.tile([32, 1], F32, tag="red")
            nc.vector.tensor_reduce(out=red[:], in_=ts2[:],
                                    axis=mybir.AxisListType.X,
                                    op=mybir.AluOpType.add)
            nc.sync.dma_start(out=CCI[:], in_=red[:])
            tc.strict_bb_all_engine_barrier()
            nc.gpsimd.collective_compute(
                "AllReduce", mybir.AluOpType.add,
                replica_groups=[list(range(NC))],
                ins=[CCI[:].opt()], outs=[CCO[:].opt()])
            tc.strict_bb_all_engine_barrier()
            ssum = wrk.tile([32, 1], F32, tag="ssum")
            nc.sync.dma_start(out=ssum[:], in_=CCO[:])
            rc32 = wrk.tile([32, 1], F32, tag="rc32")
            nc.vector.reciprocal(rc32[:], ssum[:])
            rp1 = ps.tile([1, 32], F32, tag="tp")
            nc.tensor.transpose(rp1[:], rc32[:], ident[0:32, 0:32])
            rs1 = wrk.tile([1, 32], F32, tag="rs1")
            nc.scalar.activation(rs1[:], rp1[:],
                                 mybir.ActivationFunctionType.Copy)
            ones = cst.tile([1, 128], F32)
            nc.vector.memset(ones[:], 1.0)
            rbp = ps.tile([128, D_IN], F32, tag="z")
            nc.tensor.matmul(rbp[:], ones[:], rs1[:], start=True, stop=True)
            rb = wrk.tile([128, D_IN], F32, tag="rb")
            nc.scalar.activation(rb[:], rbp[:],
                                 mybir.ActivationFunctionType.Copy)
            ot = wrk.tile([128, NT, D_IN], F32, tag="ot")
            nc.vector.tensor_tensor(
                out=ot[:], in0=e3_all[:],
                in1=rb[:][:, None, :].broadcast_to([128, NT, D_IN]),
                op=mybir.AluOpType.mult)
            nc.sync.dma_start(
                out=t_out[:].rearrange("(t p) c -> p t c", p=128), in_=ot[:])
    return nc


_CACHE = {}
LAST_EXEC_NS = None
LAST_TRACE_DIR = None


def kernel(x, edge_index, W1, a_src1, a_dst1, b1, W2, a_src2, a_dst2, b2,
           W3, a_src3, a_dst3, b3):
    x = np.asarray(x, np.float32)
    edge_index = np.asarray(edge_index)
    args = [np.asarray(a, np.float32) for a in
            (W1, a_src1, a_dst1, W2, a_src2, a_dst2, W3, a_src3, a_dst3)]
    Wt, idx_w, maskn, xT, Wc1, Wc2, Wc3, unperm = _prep(x, edge_index, *args)

    nc = bacc.Bacc(None, num_devices=NC, num_swdge_queues=4)
    nc = _build(nc, Wt)
    nc.compile()

    rowmask = (np.arange(NPP).reshape(NT, 128).T < NPC).astype(np.float32)
    rowmask = (np.arange(NT)[None, :] * 128 + np.arange(128)[:, None] < NPC).astype(np.float32)
    in_maps = []
    for k in range(NC):
        in_maps.append({
            "idxw": idx_w[k],
            "maskn": maskn[k],
            "xT": xT,
            "xT_own": np.ascontiguousarray(xT[:, k * NPP:(k + 1) * NPP]),
            "wc1": Wc1, "wc2": Wc2, "wc3": Wc3,
            "w3": args[6],
            "b1b": np.broadcast_to(np.asarray(b1, np.float32), (128, HC)).copy(),
            "b2b": np.broadcast_to(np.asarray(b2, np.float32), (128, HC)).copy(),
            "b3b": np.broadcast_to(np.asarray(b3, np.float32), (128, D_IN)).copy(),
            "ident": np.eye(128, dtype=np.float32),
            "rowmask": rowmask,
        })
    import time as _time, tempfile as _tf, os as _os
    _t0 = _time.time()
    _tdir = _os.environ.get("BASS_TRACE_DIR") or _tf.mkdtemp(prefix="bass_trace_")
    try:
        res = run_bass_kernel_spmd(nc, in_maps, core_ids=list(range(NC)),
                                   trace=True, tmpdir=_tdir)
    except Exception:
        res = run_bass_kernel_spmd(nc, in_maps, core_ids=list(range(NC)))
    global LAST_EXEC_NS, LAST_TRACE_DIR
    LAST_TRACE_DIR = _tdir
    LAST_EXEC_NS = res.exec_time_ns or int((_time.time() - _t0) * 1e9)
    outs = [res.results[k]["out"] for k in range(NC)]
    stacked = np.concatenate([o[:NPC] for o in outs], axis=0)
    full = np.zeros((N, D_IN), np.float32)
    full[unperm] = stacked
    return full



# revision 5
# speedup vs baseline: 720.9208x; 580.6072x over previous
"""GAT 3-layer (DiffusionOrderingNetwork) Trainium2 kernel, 8-core SPMD.

Strategy: nodes partitioned 8x2500 by dst; per-core ELL (degree-sorted,
per-tile width) edge layout; per-edge gathers via dma_gather 256B tokens
from an HBM node table [20480, 64] = [payload(36) | al_src(6) | al_dst(6) |
pad]; segment softmax uses a constant-shift exp (no segment max needed —
mathematically identical after normalization); messages multiplied+reduced
densely on DVE; AllGather rebuilds the table between layers.
"""

import sys

sys.path.insert(0, "/opt/trn_rl_repo")

import numpy as np
import concourse.bass as bass
import concourse.bacc as bacc
import concourse.mybir as mybir
import concourse.tile as tile
import concourse.bass_isa as bass_isa
from concourse import library_config
from concourse.bass_utils import run_bass_kernel_spmd

N = 20000
NC = 8
NPC = 2500          # nodes per core
NT = 20             # node tiles per core (128 rows each; 2560 padded rows)
NPP = NT * 128      # 2560 padded nodes per core
NTBL = NC * NPP     # 20480 table rows
H = 6
D_IN = 32
HC = 36             # heads * hidden
TW = 64             # table row width (f32) = 256B token
F32 = mybir.dt.float32
I16 = mybir.dt.int16


def _blockdiag(a):
    # a: [H, C] -> [H*C, H] with col h = a[h] at rows h*C:(h+1)*C
    Hh, C = a.shape
    out = np.zeros((Hh * C, Hh), np.float32)
    for h in range(Hh):
        out[h * C:(h + 1) * C, h] = a[h]
    return out


def _prep(x, edge_index, W1, a_src1, a_dst1, W2, a_src2, a_dst2, W3, a_src3, a_dst3):
    src = np.concatenate([edge_index[0], np.arange(N)]).astype(np.int64)
    dst = np.concatenate([edge_index[1], np.arange(N)]).astype(np.int64)

    deg = np.bincount(dst, minlength=N)
    # per-core degree-sorted node order; global_pos[n] = table row of node n
    orders = []          # per core: local pos -> global node id
    global_pos = np.zeros(N, np.int64)
    for k in range(NC):
        d = deg[k * NPC:(k + 1) * NPC]
        order = np.argsort(-d, kind="stable") + k * NPC
        orders.append(order)
        global_pos[order] = k * NPP + np.arange(NPC)

    # shared tile width schedule (max across cores)
    Wt = np.zeros(NT, np.int64)
    for k in range(NC):
        ds_ = np.sort(deg[k * NPC:(k + 1) * NPC])[::-1]
        ds_ = np.concatenate([ds_, np.zeros(NPP - NPC, np.int64)])
        Wt = np.maximum(Wt, ds_.reshape(NT, 128).max(axis=1))
    Wt = np.maximum((Wt + 3) // 4 * 4, 4).astype(np.int64)

    # CSR by dst
    sort_by_dst = np.argsort(dst, kind="stable")
    src_s = src[sort_by_dst]
    rowptr = np.zeros(N + 1, np.int64)
    np.cumsum(deg, out=rowptr[1:])

    idx_w = []   # per core: [128, 8*sum(Wt)] int16 wrapped index stream
    maskn = []   # per core: [128, sum(Wt)] f32 (0 valid, -1e4 pad)
    for k in range(NC):
        order = orders[k]
        iw_parts, mn_parts = [], []
        for t in range(NT):
            w = int(Wt[t])
            ell = np.zeros((128, w), np.int64)
            mn = np.full((128, w), -10000.0, np.float32)
            for p in range(128):
                li = t * 128 + p
                if li < NPC:
                    n = order[li]
                    e0, e1 = rowptr[n], rowptr[n + 1]
                    dd = int(e1 - e0)
                    ell[p, :dd] = global_pos[src_s[e0:e1]]
                    mn[p, :dd] = 0.0
                else:
                    mn[p, 0] = 0.0  # pad row: one live slot so denom > 0
            stream = ell.T.reshape(-1)            # slot-major: s*128+p
            iw = stream.reshape(-1, 16).T         # [16, 8w]
            iw_parts.append(np.tile(iw, (8, 1))[:128])  # replicate block to 128
            mn_parts.append(mn)
        idx_w.append(np.concatenate(iw_parts, axis=1).astype(np.int16))
        maskn.append(np.concatenate(mn_parts, axis=1))

    # x in permuted order, padded, transposed: [32, 20480]
    xp = np.zeros((NTBL, D_IN), np.float32)
    for k in range(NC):
        xp[k * NPP:k * NPP + NPC] = x[orders[k]]
    xT = np.ascontiguousarray(xp.T)

    Wc1 = np.concatenate([W1, W1 @ _blockdiag(a_src1), W1 @ _blockdiag(a_dst1)], 1)
    Wc2 = np.concatenate([W2, W2 @ _blockdiag(a_src2), W2 @ _blockdiag(a_dst2)], 1)
    I36 = np.eye(HC, dtype=np.float32)
    Wc3 = np.concatenate([I36, W3 @ _blockdiag(a_src3), W3 @ _blockdiag(a_dst3)], 1)

    unperm = np.concatenate(orders)  # row i of stacked core outputs -> node id
    return Wt, idx_w, maskn, xT, Wc1, Wc2, Wc3, unperm


def _build(nc, Wt):
    SWt = int(Wt.sum())
    CIDX = 8 * SWt

    # external inputs (per-core data passed via in_maps)
    t_idx = nc.dram_tensor("idxw", [128, CIDX], I16, kind="ExternalInput")
    t_mn = nc.dram_tensor("maskn", [128, SWt], F32, kind="ExternalInput")
    t_xT = nc.dram_tensor("xT", [D_IN, NTBL], F32, kind="ExternalInput")
    t_wc1 = nc.dram_tensor("wc1", [D_IN, 48], F32, kind="ExternalInput")
    t_wc2 = nc.dram_tensor("wc2", [HC, 48], F32, kind="ExternalInput")
    t_wc3 = nc.dram_tensor("wc3", [HC, 48], F32, kind="ExternalInput")
    t_w3 = nc.dram_tensor("w3", [HC, 192], F32, kind="ExternalInput")
    t_b1 = nc.dram_tensor("b1b", [128, HC], F32, kind="ExternalInput")
    t_b2 = nc.dram_tensor("b2b", [128, HC], F32, kind="ExternalInput")
    t_b3 = nc.dram_tensor("b3b", [128, D_IN], F32, kind="ExternalInput")
    t_rm = nc.dram_tensor("rowmask", [128, NT], F32, kind="ExternalInput")
    t_out = nc.dram_tensor("out", [NPP, D_IN], F32, kind="ExternalOutput")

    with tile.TileContext(nc) as tc:
        with (
            tc.tile_pool(name="dram", bufs=1, space="DRAM") as dram,
            tc.tile_pool(name="cst", bufs=1) as cst,
            tc.tile_pool(name="gat", bufs=4) as gat,
            tc.tile_pool(name="wrk", bufs=3) as wrk,
            tc.tile_pool(name="acc", bufs=1) as acc,
            tc.tile_pool(name="ps", bufs=2, space="PSUM") as ps,
        ):
            nc.gpsimd.load_library(library_config.mlp)
            TBL = dram.tile([NTBL, TW], F32)
            TBLS = [dram.tile([NTBL, TW], F32, addr_space="Shared", name="tbls2", tag="tbls2"),
                    dram.tile([NTBL, TW], F32, addr_space="Shared", name="tbls3", tag="tbls3")]
            BNC = dram.tile([NPP, TW], F32)
            CCI = dram.tile([32, 1], F32)
            CCO = dram.tile([32, 1], F32)

            sb_idx = cst.tile([128, CIDX], I16)
            nc.sync.dma_start(out=sb_idx[:], in_=t_idx[:])
            sb_mn = cst.tile([128, SWt], F32)
            nc.sync.dma_start(out=sb_mn[:], in_=t_mn[:])
            sb_wc = [cst.tile([D_IN, 48], F32, tag="wc0", name="wc0"),
                     cst.tile([HC, 48], F32, tag="wc1t", name="wc1t"),
                     cst.tile([HC, 48], F32, tag="wc2t", name="wc2t")]
            nc.sync.dma_start(out=sb_wc[0][:], in_=t_wc1[:])
            nc.sync.dma_start(out=sb_wc[1][:], in_=t_wc2[:])
            nc.sync.dma_start(out=sb_wc[2][:], in_=t_wc3[:])
            sb_w3 = cst.tile([HC, 192], F32)
            nc.sync.dma_start(out=sb_w3[:], in_=t_w3[:])
            sb_b = [cst.tile([128, HC], F32, tag="b0", name="b0"),
                    cst.tile([128, HC], F32, tag="b1t", name="b1t"),
                    cst.tile([128, D_IN], F32, tag="b2t", name="b2t")]
            nc.sync.dma_start(out=sb_b[0][:], in_=t_b1[:])
            nc.sync.dma_start(out=sb_b[1][:], in_=t_b2[:])
            nc.sync.dma_start(out=sb_b[2][:], in_=t_b3[:])
            t_id = nc.dram_tensor("ident", [128, 128], F32, kind="ExternalInput")
            ident = cst.tile([128, 128], F32)
            nc.sync.dma_start(out=ident[:], in_=t_id[:])
            bm20 = cst.tile([128, 1], F32)
            nc.vector.memset(bm20[:], -20.0)
            bm50 = cst.tile([128, 1], F32)
            nc.vector.memset(bm50[:], -50.0)

            # persistent per-layer state
            ald_own = [acc.tile([128, NT, H], F32, tag="ald0", name="ald0"),
                       acc.tile([128, NT, H], F32, tag="ald1", name="ald1")]
            h_all = acc.tile([128, NT, HC], F32)
            e3_all = acc.tile([128, NT, D_IN], F32)

            # ---- layer-1 table: TBL = x_perm @ Wc1, all rows ----
            for b in range(NTBL // 128):
                xblk = wrk.tile([D_IN, 128], F32, tag="xblk")
                nc.sync.dma_start(out=xblk[:], in_=t_xT[:, b * 128:(b + 1) * 128])
                pt = ps.tile([128, 48], F32, tag="tb")
                nc.tensor.matmul(pt[:], xblk[:], sb_wc[0][:],
                                 start=True, stop=True)
                sb_tb = wrk.tile([128, 48], F32, tag="tbs")
                nc.scalar.activation(sb_tb[:], pt[:],
                                     mybir.ActivationFunctionType.Copy)
                nc.sync.dma_start(out=TBL[b * 128:(b + 1) * 128, 0:48], in_=sb_tb[:])

            cur = 0  # ald_own buffer index; filled from own-block cols
            ko = None  # own rows = input xT_own? use global: own block is rank-dep
            # ald for layer 1: own rows ald = (x_own @ Wc1)[:, 42:48].
            # Own x rows are provided via xT_own input to stay SPMD-uniform.
            t_xTo = nc.dram_tensor("xT_own", [D_IN, NPP], F32, kind="ExternalInput")
            sb_xTo = cst.tile([D_IN, NPP], F32)
            nc.sync.dma_start(out=sb_xTo[:], in_=t_xTo[:])
            for t in range(NT):
                pt = ps.tile([128, 48], F32, tag="tb")
                nc.tensor.matmul(pt[:], sb_xTo[:, t * 128:(t + 1) * 128], sb_wc[0][:],
                                 start=True, stop=True)
                nc.scalar.activation(ald_own[0][:, t, :], pt[:, 42:48],
                                     mybir.ActivationFunctionType.Copy)

            # ---- layers ----
            qctr = 0
            for li in range(3):
                ioff = 0
                moff = 0
                for t in range(NT):
                    w = int(Wt[t])
                    ni = 128 * w
                    G = gat.tile([128, w, TW], F32, tag="G")
                    for c in range(0, w, 8):
                        cw = min(8, w - c)
                        cni = 128 * cw
                        nc.gpsimd.dma_gather(
                            out_ap=G[:, c:c + cw, :],
                            in_ap=(TBL if li == 0 else TBLS[li - 1])[:],
                            idxs_ap=sb_idx[:, ioff + 8 * c:ioff + 8 * (c + cw)],
                            num_idxs=cni, num_idxs_reg=cni, elem_size=TW,
                            queue_num=qctr % 4,
                        )
                        qctr += 1
                    # aldm[p,s,h] = ald_own[p,h] + maskneg[p,s]
                    aldm = wrk.tile([128, w, H], F32, tag="aldm")
                    nc.vector.tensor_tensor(
                        out=aldm[:],
                        in0=ald_own[li % 2][:, t, :][:, None, :].broadcast_to([128, w, H]),
                        in1=sb_mn[:, moff:moff + w][:, :, None].broadcast_to([128, w, H]),
                        op=mybir.AluOpType.add)
                    lg = wrk.tile([128, w, H], F32, tag="lg")
                    nc.vector.tensor_tensor(out=lg[:], in0=G[:, :, HC:HC + H],
                                            in1=aldm[:], op=mybir.AluOpType.add)
                    lgs = wrk.tile([128, w, H], F32, tag="lgs")
                    nc.vector.tensor_scalar_mul(lgs[:], lg[:], 0.2)
                    nc.vector.tensor_max(lg[:], lg[:], lgs[:])
                    ex = wrk.tile([128, w, H], F32, tag="ex")
                    nc.scalar.activation(ex[:], lg[:],
                                         mybir.ActivationFunctionType.Exp,
                                         bias=bm20[:])
                    den = wrk.tile([128, H], F32, tag="den")
                    nc.vector.tensor_reduce(
                        out=den[:], in_=ex[:].rearrange("p s h -> p h s"),
                        axis=mybir.AxisListType.X, op=mybir.AluOpType.add)
                    rd = wrk.tile([128, H], F32, tag="rd")
                    nc.vector.reciprocal(rd[:], den[:])
                    if li < 2:
                        msg = wrk.tile([128, w, H, H], F32, tag="msg")
                        nc.vector.tensor_tensor(
                            out=msg[:],
                            in0=ex[:][:, :, :, None].broadcast_to([128, w, H, H]),
                            in1=G[:, :, 0:HC].rearrange("p s (h c) -> p s h c", h=H),
                            op=mybir.AluOpType.mult)
                        agg = wrk.tile([128, HC], F32, tag="agg")
                        nc.vector.tensor_reduce(
                            out=agg[:], in_=msg[:].rearrange("p s h c -> p (h c) s"),
                            axis=mybir.AxisListType.X, op=mybir.AluOpType.add)
                        hp = wrk.tile([128, HC], F32, tag="hp")
                        nc.vector.tensor_tensor(
                            out=hp[:].rearrange("p (h c) -> p h c", h=H),
                            in0=agg[:].rearrange("p (h c) -> p h c", h=H),
                            in1=rd[:][:, :, None].broadcast_to([128, H, H]),
                            op=mybir.AluOpType.mult)
                        nc.vector.tensor_tensor(out=hp[:], in0=hp[:], in1=sb_b[li][:],
                                                op=mybir.AluOpType.add)
                        nc.scalar.activation(h_all[:, t, :], hp[:],
                                             mybir.ActivationFunctionType.Relu)
                    else:
                        agg3 = wrk.tile([128, H, HC], F32, tag="agg3")
                        for h in range(H):
                            m3 = wrk.tile([128, w, HC], F32, tag="m3")
                            nc.vector.tensor_tensor(
                                out=m3[:],
                                in0=ex[:, :, h][:, :, None].broadcast_to([128, w, HC]),
                                in1=G[:, :, 0:HC],
                                op=mybir.AluOpType.mult)
                            nc.vector.tensor_reduce(
                                out=agg3[:, h, :],
                                in_=m3[:].rearrange("p s c -> p c s"),
                                axis=mybir.AxisListType.X, op=mybir.AluOpType.add)
                        nc.vector.tensor_tensor(
                            out=agg3[:],
                            in0=agg3[:],
                            in1=rd[:][:, :, None].broadcast_to([128, H, HC]),
                            op=mybir.AluOpType.mult)
                        zp = ps.tile([128, D_IN], F32, tag="z")
                        for h in range(H):
                            tp = ps.tile([36, 128], F32, tag="tp")
                            nc.tensor.transpose(tp[:], agg3[:, h, :], ident[:])
                            ts = wrk.tile([36, 128], F32, tag="ts")
                            nc.scalar.activation(ts[:], tp[:],
                                                 mybir.ActivationFunctionType.Copy)
                            nc.tensor.matmul(zp[:], ts[:],
                                             sb_w3[:, h * 32:(h + 1) * 32],
                                             start=(h == 0), stop=(h == 5))
                        zs = wrk.tile([128, D_IN], F32, tag="zs")
                        nc.vector.tensor_scalar_mul(zs[:], zp[:], 1.0 / 6.0)
                        nc.vector.tensor_tensor(out=zs[:], in0=zs[:], in1=sb_b[2][:],
                                                op=mybir.AluOpType.add)
                        nc.scalar.activation(e3_all[:, t, :], zs[:],
                                             mybir.ActivationFunctionType.Exp,
                                             bias=bm50[:])
                    ioff += 8 * w
                    moff += w

                if li < 2:
                    # table build for next layer + AllGather
                    for t in range(NT):
                        tp = ps.tile([36, 128], F32, tag="tp")
                        nc.tensor.transpose(tp[:], h_all[:, t, :], ident[:])
                        ts = wrk.tile([36, 128], F32, tag="ts")
                        nc.scalar.activation(ts[:], tp[:],
                                             mybir.ActivationFunctionType.Copy)
                        pt = ps.tile([128, 48], F32, tag="tb")
                        nc.tensor.matmul(pt[:], ts[:], sb_wc[li + 1][:],
                                         start=True, stop=True)
                        nc.scalar.activation(ald_own[(li + 1) % 2][:, t, :],
                                             pt[:, 42:48],
                                             mybir.ActivationFunctionType.Copy)
                        sb_tb = wrk.tile([128, 48], F32, tag="tbs")
                        nc.scalar.activation(sb_tb[:], pt[:],
                                             mybir.ActivationFunctionType.Copy)
                        nc.sync.dma_start(out=BNC[t * 128:(t + 1) * 128, 0:48],
                                          in_=sb_tb[:])
                    tc.strict_bb_all_engine_barrier()
                    nc.gpsimd.collective_compute(
                        "AllGather", mybir.AluOpType.bypass,
                        replica_groups=[list(range(NC))],
                        ins=[BNC[:].opt()], outs=[TBLS[li][:].opt()])
                    tc.strict_bb_all_engine_barrier()

            # ---- global softmax over nodes ----
            sb_rm = cst.tile([128, NT], F32)
            nc.sync.dma_start(out=sb_rm[:], in_=t_rm[:])
            nc.vector.tensor_tensor(
                out=e3_all[:], in0=e3_all[:],
                in1=sb_rm[:][:, :, None].broadcast_to([128, NT, D_IN]),
                op=mybir.AluOpType.mult)
            s0 = wrk.tile([128, D_IN], F32, tag="s0")
            nc.vector.tensor_reduce(out=s0[:],
                                    in_=e3_all[:].rearrange("p t c -> p c t"),
                                    axis=mybir.AxisListType.X, op=mybir.AluOpType.add)
            tp2 = ps.tile([32, 128], F32, tag="tp")
            nc.tensor.transpose(tp2[:], s0[:], ident[:])
            ts2 = wrk.tile([32, 128], F32, tag="ts2")
            nc.scalar.activation(ts2[:], tp2[:],
                                 mybir.ActivationFunctionType.Copy)
            red = wrk.tile([32, 1], F32, tag="red")
            nc.vector.tensor_reduce(out=red[:], in_=ts2[:],
                                    axis=mybir.AxisListType.X,
                                    op=mybir.AluOpType.add)
            nc.sync.dma_start(out=CCI[:], in_=red[:])
            tc.strict_bb_all_engine_barrier()
            nc.gpsimd.collective_compute(
                "AllReduce", mybir.AluOpType.add,
                replica_groups=[list(range(NC))],
                ins=[CCI[:].opt()], outs=[CCO[:].opt()])
            tc.strict_bb_all_engine_barrier()
            ssum = wrk.tile([32, 1], F32, tag="ssum")
            nc.sync.dma_start(out=ssum[:], in_=CCO[:])
            rc32 = wrk.tile([32, 1], F32, tag="rc32")
            nc.vector.reciprocal(rc32[:], ssum[:])
            rp1 = ps.tile([1, 32], F32, tag="tp")
            nc.tensor.transpose(rp1[:], rc32[:], ident[0:32, 0:32])
            rs1 = wrk.tile([1, 32], F32, tag="rs1")
            nc.scalar.activation(rs1[:], rp1[:],
                                 mybir.ActivationFunctionType.Copy)
            ones = cst.tile([1, 128], F32)
            nc.vector.memset(ones[:], 1.0)
            rbp = ps.tile([128, D_IN], F32, tag="z")
            nc.tensor.matmul(rbp[:], ones[:], rs1[:], start=True, stop=True)
            rb = wrk.tile([128, D_IN], F32, tag="rb")
            nc.scalar.activation(rb[:], rbp[:],
                                 mybir.ActivationFunctionType.Copy)
            ot = wrk.tile([128, NT, D_IN], F32, tag="ot")
            nc.vector.tensor_tensor(
                out=ot[:], in0=e3_all[:],
                in1=rb[:][:, None, :].broadcast_to([128, NT, D_IN]),
                op=mybir.AluOpType.mult)
            nc.sync.dma_start(
                out=t_out[:].rearrange("(t p) c -> p t c", p=128), in_=ot[:])
    return nc


_CACHE = {}
LAST_EXEC_NS = None
LAST_TRACE_DIR = None


def _run_timed(nc, in_maps, n_iter=32):
    """Execute the compiled SPMD kernel and measure HW execution time.

    No NTFF profiling hook exists under this axon tunnel, so
    neuron-profile exec_time_ns is unavailable. Closest honest proxy:
    pre-stage all inputs in device HBM (NTFF exec time excludes host
    transfers too), then time n_iter back-to-back executions of the
    compiled NEFF on all 8 cores and report the per-iteration mean.
    The one-time ~85 ms axon RPC latency is excluded via a warmup run;
    outputs are taken from the warmup execution.
    """
    import time
    import jax
    from jax.sharding import Mesh, PartitionSpec, NamedSharding
    from jax.experimental.shard_map import shard_map
    import concourse.bass2jax as b2j

    b2j.install_neuronx_cc_hook()
    partition_name = nc.partition_id_tensor.name if nc.partition_id_tensor else None
    in_names, out_names, out_avals, out_shapes = [], [], [], []
    for alloc in nc.m.functions[0].allocations:
        if not isinstance(alloc, mybir.MemoryLocationSet):
            continue
        name = alloc.memorylocations[0].name
        if alloc.kind == "ExternalInput":
            if name != partition_name:
                in_names.append(name)
        elif alloc.kind == "ExternalOutput":
            out_names.append(name)
            shape = tuple(alloc.tensor_shape)
            dtype = mybir.dt.np(alloc.dtype)
            out_avals.append(jax.core.ShapedArray(shape, dtype))
            out_shapes.append((shape, dtype))
    n_params = len(in_names)
    n_outs = len(out_avals)
    in_names.extend(out_names)
    if partition_name is not None:
        in_names.append(partition_name)
    donate = tuple(range(n_params, n_params + n_outs))

    def _body(*a):
        operands = list(a)
        if partition_name is not None:
            operands.append(b2j.partition_id_tensor())
        outs = b2j._bass_exec_p.bind(
            *operands, out_avals=tuple(out_avals), in_names=tuple(in_names),
            out_names=tuple(out_names), lowering_input_output_aliases=(),
            sim_require_finite=True, sim_require_nnan=True, nc=nc)
        return tuple(outs)

    devices = jax.devices()[:NC]
    mesh = Mesh(np.asarray(devices), ("core",))
    sh = NamedSharding(mesh, PartitionSpec("core"))
    sharded = jax.jit(
        shard_map(_body, mesh=mesh,
                  in_specs=(PartitionSpec("core"),) * (n_params + n_outs),
                  out_specs=(PartitionSpec("core"),) * n_outs,
                  check_rep=False),
        donate_argnums=donate, keep_unused=True)
    concat_in = [np.concatenate([np.asarray(m[name]) for m in in_maps], axis=0)
                 for name in in_names[:n_params]]
    zeros = [np.zeros((NC * s[0], *s[1:]), d) for s, d in out_shapes]
    compiled = sharded.lower(*concat_in, *zeros).compile()

    dev_in = [jax.device_put(a, sh) for a in concat_in]
    dz_warm = [jax.device_put(z, sh) for z in zeros]
    dz_sets = [[jax.device_put(z, sh) for z in zeros] for _ in range(n_iter)]
    jax.block_until_ready(dev_in)
    jax.block_until_ready(dz_warm)
    jax.block_until_ready(dz_sets)

    warm = compiled(*dev_in, *dz_warm)
    jax.block_until_ready(warm)

    t0 = time.time()
    res = None
    for i in range(n_iter):
        res = compiled(*dev_in, *dz_sets[i])
    jax.block_until_ready(res)
    t1 = time.time()
    exec_ns = int((t1 - t0) / n_iter * 1e9)

    outs = []
    for c in range(NC):
        m = {}
        for i, name in enumerate(out_names):
            shape, _ = out_shapes[i]
            m[name] = np.asarray(warm[i]).reshape(NC, *shape)[c]
        outs.append(m["out"])
    return outs, exec_ns


def kernel(x, edge_index, W1, a_src1, a_dst1, b1, W2, a_src2, a_dst2, b2,
           W3, a_src3, a_dst3, b3):
    x = np.asarray(x, np.float32)
    edge_index = np.asarray(edge_index)
    args = [np.asarray(a, np.float32) for a in
            (W1, a_src1, a_dst1, W2, a_src2, a_dst2, W3, a_src3, a_dst3)]
    Wt, idx_w, maskn, xT, Wc1, Wc2, Wc3, unperm = _prep(x, edge_index, *args)

    nc = bacc.Bacc(None, num_devices=NC, num_swdge_queues=4)
    nc = _build(nc, Wt)
    nc.compile()

    rowmask = (np.arange(NPP).reshape(NT, 128).T < NPC).astype(np.float32)
    rowmask = (np.arange(NT)[None, :] * 128 + np.arange(128)[:, None] < NPC).astype(np.float32)
    in_maps = []
    for k in range(NC):
        in_maps.append({
            "idxw": idx_w[k],
            "maskn": maskn[k],
            "xT": xT,
            "xT_own": np.ascontiguousarray(xT[:, k * NPP:(k + 1) * NPP]),
            "wc1": Wc1, "wc2": Wc2, "wc3": Wc3,
            "w3": args[6],
            "b1b": np.broadcast_to(np.asarray(b1, np.float32), (128, HC)).copy(),
            "b2b": np.broadcast_to(np.asarray(b2, np.float32), (128, HC)).copy(),
            "b3b": np.broadcast_to(np.asarray(b3, np.float32), (128, D_IN)).copy(),
            "ident": np.eye(128, dtype=np.float32),
            "rowmask": rowmask,
        })
    global LAST_EXEC_NS
    try:
        outs, LAST_EXEC_NS = _run_timed(nc, in_maps)
    except Exception:
        import time as _time
        _t0 = _time.time()
        res = run_bass_kernel_spmd(nc, in_maps, core_ids=list(range(NC)))
        LAST_EXEC_NS = res.exec_time_ns or int((_time.time() - _t0) * 1e9)
        outs = [res.results[k]["out"] for k in range(NC)]
    stacked = np.concatenate([o[:NPC] for o in outs], axis=0)
    full = np.zeros((N, D_IN), np.float32)
    full[unperm] = stacked
    return full



# revision 13
# speedup vs baseline: 790.6126x; 1.0967x over previous
"""GAT 3-layer (DiffusionOrderingNetwork) Trainium2 kernel, 8-core SPMD.

Design (v2):
- Nodes partitioned 8x2500 by dst; per-core ELL (degree-sorted, per-tile
  width) edge layout; per-edge gathers via gpsimd dma_gather from an fp16
  HBM node table. Layers 1/2: 256B rows = [xw(36, (c,h)-major) | al_src(6)
  | pad]; layer 3 folds W3 into the table: 512B rows = [xw3/6 (192,
  (c,h)-major) | al_src3(6) | pad].
- Each core builds only its own 2560-row block (PE matmul vs fused
  [W | W@bd(a_src) | W@bd(a_dst)]), AllGathers the compact fp16 rows,
  then locally expands into the 256B/512B-strided gather table.
- Segment softmax uses a constant-shift exp (exp(lg-20), f32) -- no
  segment max needed; alpha = ex * (1/den) cast to fp16 before the
  payload multiply so the big DVE ops run in the 2x fp16 mode.
- Padding handled by two special table rows per core block: Z0 (zeros)
  and ZNEG (al_src = -1e4): pad slots gather ZNEG (ex -> 0), pad dst
  rows gather Z0 once so den > 0. No mask tensor at all.
- Layer 3: alpha-weighted sum of xw3 directly (outer product over
  (c=32, h=6)), head-mean = packed reduce; no per-head PE loop.
- Final global softmax over nodes via ones-matmul partition reduction +
  AllReduce + ones-matmul broadcast.
"""

import sys

sys.path.insert(0, "/opt/trn_rl_repo")

import numpy as np
import concourse.bass as bass
import concourse.bacc as bacc
import concourse.mybir as mybir
import concourse.tile as tile
import concourse.bass_isa as bass_isa
from concourse import library_config
from concourse.bass_utils import run_bass_kernel_spmd

N = 20000
NC = 8
NPC = 2500          # nodes per core
NT = 20             # node tiles per core (128 rows each; 2560 padded rows)
NPP = NT * 128      # 2560 padded rows per core
NTBL = NC * NPP     # 20480 table rows
H = 6
D_IN = 32
HC = 36             # heads * hidden
F32 = mybir.dt.float32
F16 = mybir.dt.float16
I16 = mybir.dt.int16
AF = mybir.ActivationFunctionType

CHUNK = 8           # gather chunk in ELL slots (128*CHUNK tokens per call)
SCRATCH = 32768     # SWDGE descriptor ring carveout (descs = SCRATCH//16)

# (c,h)-major permutations: new col j' = c*H + h holds old col h*C + c
PERM36 = np.array([(j % H) * 6 + j // H for j in range(36)])     # C=6
PERM192 = np.array([(j % H) * 32 + j // H for j in range(192)])  # C=32


def _blockdiag(a):
    # a: [H, C] -> [H*C, H] with col h = a[h] at rows h*C:(h+1)*C
    Hh, C = a.shape
    out = np.zeros((Hh * C, Hh), np.float32)
    for h in range(Hh):
        out[h * C:(h + 1) * C, h] = a[h]
    return out


def _prep(x, edge_index, W1, a_src1, a_dst1, W2, a_src2, a_dst2, W3, a_src3, a_dst3):
    src = np.concatenate([edge_index[0], np.arange(N)]).astype(np.int64)
    dst = np.concatenate([edge_index[1], np.arange(N)]).astype(np.int64)

    deg = np.bincount(dst, minlength=N)
    orders = []          # per core: local pos -> global node id
    global_pos = np.zeros(N, np.int64)
    for k in range(NC):
        d = deg[k * NPC:(k + 1) * NPC]
        order = np.argsort(-d, kind="stable") + k * NPC
        orders.append(order)
        global_pos[order] = k * NPP + np.arange(NPC)

    # shared tile width schedule (max across cores)
    Wt = np.zeros(NT, np.int64)
    for k in range(NC):
        ds_ = np.sort(deg[k * NPC:(k + 1) * NPC])[::-1]
        ds_ = np.concatenate([ds_, np.zeros(NPP - NPC, np.int64)])
        Wt = np.maximum(Wt, ds_.reshape(NT, 128).max(axis=1))
    Wt = np.maximum((Wt + 3) // 4 * 4, 4).astype(np.int64)

    # CSR by dst
    sort_by_dst = np.argsort(dst, kind="stable")
    src_s = src[sort_by_dst]
    rowptr = np.zeros(N + 1, np.int64)
    np.cumsum(deg, out=rowptr[1:])

    idx_w = []   # per core: [16, 8*sum(Wt)] int16 wrapped index stream
    for k in range(NC):
        order = orders[k]
        z0 = k * NPP + NPC        # all-zero row
        zneg = k * NPP + NPC + 1  # al_src = -1e4 row
        iw_parts = []
        for t in range(NT):
            w = int(Wt[t])
            ell = np.full((128, w), zneg, np.int64)
            for p in range(128):
                li = t * 128 + p
                if li < NPC:
                    n = order[li]
                    e0, e1 = rowptr[n], rowptr[n + 1]
                    dd = int(e1 - e0)
                    ell[p, :dd] = global_pos[src_s[e0:e1]]
                else:
                    ell[p, 0] = z0  # pad row: one live slot so den > 0
            stream = ell.T.reshape(-1)            # slot-major: s*128+p
            iw_parts.append(stream.reshape(-1, 16).T)   # [16, 8w]
        idx_w.append(np.concatenate(iw_parts, axis=1).astype(np.int16))

    # x in permuted order, padded, transposed: [32, NPP] f16 per core
    xp = np.zeros((NTBL, D_IN), np.float32)
    for k in range(NC):
        xp[k * NPP:k * NPP + NPC] = x[orders[k]]
    xTo = [np.ascontiguousarray(xp[k * NPP:(k + 1) * NPP].T).astype(np.float16)
           for k in range(NC)]

    Wc1 = np.concatenate(
        [W1[:, PERM36], W1 @ _blockdiag(a_src1), W1 @ _blockdiag(a_dst1)], 1)
    W2r = W2[PERM36, :]
    Wc2 = np.concatenate(
        [W2r[:, PERM36], W2r @ _blockdiag(a_src2), W2r @ _blockdiag(a_dst2)], 1)
    W3r = W3[PERM36, :]
    Wc3 = np.concatenate(
        [W3r[:, PERM192] / 6.0, W3r @ _blockdiag(a_src3), W3r @ _blockdiag(a_dst3)], 1)

    unperm = np.concatenate(orders)  # row i of stacked core outputs -> node id
    return Wt, idx_w, xTo, Wc1.astype(np.float16), Wc2.astype(np.float16), \
        Wc3.astype(np.float16), unperm


def _build(nc, Wt):
    SWt = int(Wt.sum())
    CIDX = 8 * SWt

    t_idx = nc.dram_tensor("idxw", [16, CIDX], I16, kind="ExternalInput")
    t_xTo = nc.dram_tensor("xT_own", [D_IN, NPP], F16, kind="ExternalInput")
    t_wc1 = nc.dram_tensor("wc1", [D_IN, 48], F16, kind="ExternalInput")
    t_wc2 = nc.dram_tensor("wc2", [HC, 48], F16, kind="ExternalInput")
    t_wc3 = nc.dram_tensor("wc3", [HC, 204], F16, kind="ExternalInput")
    t_b1 = nc.dram_tensor("b1b", [128, HC], F32, kind="ExternalInput")
    t_b2 = nc.dram_tensor("b2b", [128, HC], F32, kind="ExternalInput")
    t_b3 = nc.dram_tensor("b3b", [128, D_IN], F32, kind="ExternalInput")
    t_id = nc.dram_tensor("ident", [128, 128], F16, kind="ExternalInput")
    t_rm = nc.dram_tensor("rowmask", [128, NT], F32, kind="ExternalInput")
    t_out = nc.dram_tensor("out", [NPP, D_IN], F32, kind="ExternalOutput")

    with tile.TileContext(nc) as tc:
        with (
            tc.tile_pool(name="dram", bufs=1, space="DRAM") as dram,
            tc.tile_pool(name="cst", bufs=1) as cst,
            tc.tile_pool(name="gat", bufs=2) as gat,
            tc.tile_pool(name="stg", bufs=2) as stg,
            tc.tile_pool(name="wrkb", bufs=2) as wrkb,
            tc.tile_pool(name="wrk", bufs=3) as wrk,
            tc.tile_pool(name="acc", bufs=1) as acc,
            tc.tile_pool(name="ps", bufs=2, space="PSUM") as ps,
        ):
            nc.gpsimd.load_library(library_config.mlp)
            TBL12 = dram.tile([NTBL, 128], F16)
            TBL3 = dram.tile([NTBL, 256], F16, name="tbl3", tag="tbl3")
            BNC12 = dram.tile([NPP, 42], F16, name="bnc12", tag="bnc12")
            BNC3 = dram.tile([NPP, 198], F16, name="bnc3", tag="bnc3")
            TBLS12 = [dram.tile([NTBL, 42], F16, addr_space="Shared",
                                name="tbls1", tag="tbls1"),
                      dram.tile([NTBL, 42], F16, addr_space="Shared",
                                name="tbls2", tag="tbls2")]
            TBLS3 = dram.tile([NTBL, 198], F16, addr_space="Shared",
                              name="tbls3", tag="tbls3")
            CCI = dram.tile([1, 32], F32, name="cci", tag="cci")
            CCO = dram.tile([1, 32], F32, name="cco", tag="cco")

            # ---- constants ----
            sb_idx = cst.tile([128, CIDX], I16)
            for r in range(8):
                nc.sync.dma_start(out=sb_idx[16 * r:16 * (r + 1), :], in_=t_idx[:])
            sb_xTo = cst.tile([D_IN, NPP], F16)
            nc.sync.dma_start(out=sb_xTo[:], in_=t_xTo[:])
            sb_wc = [cst.tile([D_IN, 48], F16, tag="wc0", name="wc0"),
                     cst.tile([HC, 48], F16, tag="wc1t", name="wc1t"),
                     cst.tile([HC, 204], F16, tag="wc2t", name="wc2t")]
            nc.sync.dma_start(out=sb_wc[0][:], in_=t_wc1[:])
            nc.sync.dma_start(out=sb_wc[1][:], in_=t_wc2[:])
            nc.sync.dma_start(out=sb_wc[2][:], in_=t_wc3[:])
            sb_b = [cst.tile([128, HC], F32, tag="b0", name="b0"),
                    cst.tile([128, HC], F32, tag="b1t", name="b1t"),
                    cst.tile([128, D_IN], F32, tag="b2t", name="b2t")]
            nc.sync.dma_start(out=sb_b[0][:], in_=t_b1[:])
            nc.sync.dma_start(out=sb_b[1][:], in_=t_b2[:])
            nc.sync.dma_start(out=sb_b[2][:], in_=t_b3[:])
            ident = cst.tile([128, 128], F16)
            nc.sync.dma_start(out=ident[:], in_=t_id[:])
            sb_rm = cst.tile([128, NT], F32)
            nc.sync.dma_start(out=sb_rm[:], in_=t_rm[:])
            ones_r = cst.tile([1, 128], F32, tag="ones_r", name="ones_r")
            nc.vector.memset(ones_r[:], 1.0)
            ones_c = cst.tile([128, 1], F32, tag="ones_c", name="ones_c")
            nc.vector.memset(ones_c[:], 1.0)
            zrow = cst.tile([1, 256], F16, tag="zrow", name="zrow")
            nc.vector.memset(zrow[:], 0.0)
            nrow12 = cst.tile([1, 42], F16, tag="nrow12", name="nrow12")
            nc.vector.memset(nrow12[:], 0.0)
            nc.vector.memset(nrow12[:, 36:42], -10000.0)
            nrow3 = cst.tile([1, 198], F16, tag="nrow3", name="nrow3")
            nc.vector.memset(nrow3[:], 0.0)
            nc.vector.memset(nrow3[:, 192:198], -10000.0)
            bm20 = cst.tile([128, 1], F32, tag="bm20", name="bm20")
            nc.vector.memset(bm20[:], -20.0)
            bm50 = cst.tile([128, 1], F32, tag="bm50", name="bm50")
            nc.vector.memset(bm50[:], -50.0)

            # persistent per-layer state
            ald_own = [acc.tile([128, NT, H], F16, tag="ald0", name="ald0"),
                       acc.tile([128, NT, H], F16, tag="ald1", name="ald1")]
            h_all = acc.tile([128, NT, HC], F16)
            e3_all = acc.tile([128, NT, D_IN], F32)

            def build_tile(li, t, lhsT):
                # matmul vs fused Wc -> table row block + own al_dst
                wcols = 204 if li == 2 else 48
                pw = 198 if li == 2 else 42
                pt = ps.tile([128, 204], F32, tag="tb")
                nc.tensor.matmul(pt[:, 0:wcols], lhsT, sb_wc[li][:],
                                 start=True, stop=True)
                nc.scalar.activation(ald_own[li % 2][:, t, :],
                                     pt[:, pw:pw + H], AF.Copy)
                tb = wrk.tile([128, 198], F16, tag="tbs")
                nc.scalar.activation(tb[:, 0:pw], pt[:, 0:pw], AF.Copy)
                bnc = BNC3 if li == 2 else BNC12
                nc.sync.dma_start(out=bnc[t * 128:(t + 1) * 128, 0:pw],
                                  in_=tb[:, 0:pw])

            # ---- initial (layer-1) table from own x rows ----
            for t in range(NT):
                build_tile(0, t, sb_xTo[:, t * 128:(t + 1) * 128])
            nc.sync.dma_start(out=BNC12[NPC:NPC + 1, :], in_=zrow[:, 0:42])
            nc.sync.dma_start(out=BNC12[NPC + 1:NPC + 2, :], in_=nrow12[:])

            def allgather_expand(li):
                tbls = TBLS3 if li == 2 else TBLS12[li]
                bnc = BNC3 if li == 2 else BNC12
                tbl = TBL3 if li == 2 else TBL12
                pw = 198 if li == 2 else 42
                tw = 256 if li == 2 else 128
                tc.strict_bb_all_engine_barrier()
                nc.gpsimd.collective_compute(
                    "AllGather", mybir.AluOpType.bypass,
                    replica_groups=[list(range(NC))],
                    ins=[bnc[:].opt()], outs=[tbls[:].opt()])
                tc.strict_bb_all_engine_barrier()
                # expand compact rows into the 256B/512B-strided gather table
                nblk = NTBL // 128          # 160 row-blocks of 128
                step = 40                   # blocks per staging chunk
                for b0 in range(0, nblk, step):
                    st = stg.tile([128, 8192], F16, tag="stage")
                    sv = st[:, 0:step * pw].rearrange("p (b c) -> p b c", c=pw)
                    nc.sync.dma_start(
                        out=sv,
                        in_=tbls[b0 * 128:(b0 + step) * 128, :]
                        .rearrange("(b p) c -> p b c", p=128))
                    nc.sync.dma_start(
                        out=tbl[b0 * 128:(b0 + step) * 128, :]
                        .rearrange("(b p) c -> p b c", p=128)[:, :, 0:pw],
                        in_=sv)

            allgather_expand(0)

            # ---- layers ----
            qctr = 0
            for li in range(3):
                tbl = TBL3 if li == 2 else TBL12
                el = 256 if li == 2 else 128
                pw = 192 if li == 2 else 36
                ioff = 0
                for t in range(NT):
                    w = int(Wt[t])
                    GU = gat.tile([128, 15360], F16, tag="G")
                    G = GU.rearrange("p (s e) -> p s e", e=el)
                    for c in range(0, w, CHUNK):
                        cw = min(CHUNK, w - c)
                        cni = 128 * cw
                        nc.gpsimd.dma_gather(
                            out_ap=G[:, c:c + cw, :],
                            in_ap=tbl[:],
                            idxs_ap=sb_idx[:, ioff + 8 * c:ioff + 8 * (c + cw)],
                            num_idxs=cni, num_idxs_reg=cni, elem_size=el,
                            queue_num=qctr % 4,
                        )
                        qctr += 1
                    # attention: lg = al_src[src] + al_dst[dst]
                    lg = wrk.tile([128, 60, H], F16, tag="lg")
                    nc.vector.tensor_tensor(
                        out=lg[:, 0:w, :], in0=G[:, 0:w, pw:pw + H],
                        in1=ald_own[li % 2][:, t, :][:, None, :]
                        .broadcast_to([128, w, H]),
                        op=mybir.AluOpType.add)
                    lr = wrk.tile([128, 60, H], F16, tag="lr")
                    nc.scalar.activation(lr[:, 0:w, :], lg[:, 0:w, :],
                                         AF.Lrelu, alpha=0.2)
                    ex = wrk.tile([128, 60, H], F32, tag="ex")
                    nc.scalar.activation(ex[:, 0:w, :], lr[:, 0:w, :],
                                         AF.Exp, bias=bm20[:])
                    den = wrk.tile([128, H], F32, tag="den")
                    nc.vector.tensor_reduce(
                        out=den[:], in_=ex[:, 0:w, :].rearrange("p s h -> p h s"),
                        axis=mybir.AxisListType.X, op=mybir.AluOpType.add)
                    rd = wrk.tile([128, H], F32, tag="rd")
                    nc.vector.reciprocal(rd[:], den[:])
                    al16 = wrk.tile([128, 60, H], F16, tag="al16")
                    nc.vector.tensor_tensor(
                        out=al16[:, 0:w, :], in0=ex[:, 0:w, :],
                        in1=rd[:][:, None, :].broadcast_to([128, w, H]),
                        op=mybir.AluOpType.mult)
                    # alpha-weighted payload aggregation ((c,h)-major)
                    cdim = pw // H
                    msg = wrkb.tile([128, 60, 192], F16, tag="msg")
                    nc.vector.tensor_tensor(
                        out=msg[:, 0:w, 0:pw]
                        .rearrange("p s (c h) -> p s c h", h=H),
                        in0=G[:, 0:w, 0:pw]
                        .rearrange("p s (c h) -> p s c h", h=H),
                        in1=al16[:, 0:w, :][:, :, None, :]
                        .broadcast_to([128, w, cdim, H]),
                        op=mybir.AluOpType.mult)
                    agg = wrk.tile([128, 192], F32, tag="agg")
                    nc.vector.tensor_reduce(
                        out=agg[:, 0:pw],
                        in_=msg[:, 0:w, 0:pw].rearrange("p s j -> p j s"),
                        axis=mybir.AxisListType.X, op=mybir.AluOpType.add)
                    if li < 2:
                        hp = wrk.tile([128, HC], F32, tag="hp")
                        nc.vector.tensor_tensor(out=hp[:], in0=agg[:, 0:HC],
                                                in1=sb_b[li][:],
                                                op=mybir.AluOpType.add)
                        nc.scalar.activation(h_all[:, t, :], hp[:], AF.Relu)
                        # build next layer's table rows for this tile
                        tp = ps.tile([HC, 128], F16, tag="tp")
                        nc.tensor.transpose(tp[:], h_all[:, t, :], ident[:])
                        ts16 = wrk.tile([HC, 128], F16, tag="ts16")
                        nc.scalar.activation(ts16[:], tp[:], AF.Copy)
                        build_tile(li + 1, t, ts16[:])
                    else:
                        zs = wrk.tile([128, D_IN], F32, tag="zs")
                        nc.vector.tensor_reduce(
                            out=zs[:],
                            in_=agg[:].rearrange("p (c h) -> p c h", h=H),
                            axis=mybir.AxisListType.X, op=mybir.AluOpType.add)
                        nc.vector.tensor_tensor(out=zs[:], in0=zs[:],
                                                in1=sb_b[2][:],
                                                op=mybir.AluOpType.add)
                        nc.scalar.activation(e3_all[:, t, :], zs[:],
                                             AF.Exp, bias=bm50[:])
                    ioff += 8 * w

                if li < 2:
                    bnc = BNC3 if li == 1 else BNC12
                    pwb = 198 if li == 1 else 42
                    nrow = nrow3 if li == 1 else nrow12
                    nc.sync.dma_start(out=bnc[NPC:NPC + 1, 0:pwb],
                                      in_=zrow[:, 0:pwb])
                    nc.sync.dma_start(out=bnc[NPC + 1:NPC + 2, 0:pwb],
                                      in_=nrow[:])
                    allgather_expand(li + 1)

            # ---- global softmax over nodes ----
            nc.vector.tensor_tensor(
                out=e3_all[:], in0=e3_all[:],
                in1=sb_rm[:][:, :, None].broadcast_to([128, NT, D_IN]),
                op=mybir.AluOpType.mult)
            s0 = wrk.tile([128, D_IN], F32, tag="s0")
            nc.vector.tensor_reduce(out=s0[:],
                                    in_=e3_all[:].rearrange("p t c -> p c t"),
                                    axis=mybir.AxisListType.X,
                                    op=mybir.AluOpType.add)
            sp = ps.tile([1, D_IN], F32, tag="sp")
            nc.tensor.matmul(sp[:], ones_c[:], s0[:], start=True, stop=True)
            red = wrk.tile([1, D_IN], F32, tag="red")
            nc.scalar.activation(red[:], sp[:], AF.Copy)
            nc.sync.dma_start(out=CCI[:], in_=red[:])
            tc.strict_bb_all_engine_barrier()
            nc.gpsimd.collective_compute(
                "AllReduce", mybir.AluOpType.add,
                replica_groups=[list(range(NC))],
                ins=[CCI[:].opt()], outs=[CCO[:].opt()])
            tc.strict_bb_all_engine_barrier()
            ssum = wrk.tile([1, D_IN], F32, tag="ssum")
            nc.sync.dma_start(out=ssum[:], in_=CCO[:])
            rc = wrk.tile([1, D_IN], F32, tag="rc")
            nc.vector.reciprocal(rc[:], ssum[:])
            rbp = ps.tile([128, D_IN], F32, tag="rbp")
            nc.tensor.matmul(rbp[:], ones_r[:], rc[:], start=True, stop=True)
            rb = wrk.tile([128, D_IN], F32, tag="rb")
            nc.scalar.activation(rb[:], rbp[:], AF.Copy)
            ot = wrk.tile([128, NT, D_IN], F32, tag="ot")
            nc.vector.tensor_tensor(
                out=ot[:], in0=e3_all[:],
                in1=rb[:][:, None, :].broadcast_to([128, NT, D_IN]),
                op=mybir.AluOpType.mult)
            nc.sync.dma_start(
                out=t_out[:].rearrange("(t p) c -> p t c", p=128), in_=ot[:])
    return nc


_CACHE = {}
LAST_EXEC_NS = None
LAST_TRACE_DIR = None


def _run_timed(nc, in_maps, n_iter=32):
    """Execute the compiled SPMD kernel and measure HW execution time.

    No NTFF profiling hook exists under this axon tunnel, so
    neuron-profile exec_time_ns is unavailable. Closest honest proxy:
    pre-stage all inputs in device HBM (NTFF exec time excludes host
    transfers too), then time n_iter back-to-back executions of the
    compiled NEFF on all 8 cores and report the per-iteration mean.
    The one-time ~85 ms axon RPC latency is excluded via a warmup run;
    outputs are taken from the warmup execution.
    """
    import time
    import jax
    from jax.sharding import Mesh, PartitionSpec, NamedSharding
    from jax.experimental.shard_map import shard_map
    import concourse.bass2jax as b2j

    b2j.install_neuronx_cc_hook()
    partition_name = nc.partition_id_tensor.name if nc.partition_id_tensor else None
    in_names, out_names, out_avals, out_shapes = [], [], [], []
    for alloc in nc.m.functions[0].allocations:
        if not isinstance(alloc, mybir.MemoryLocationSet):
            continue
        name = alloc.memorylocations[0].name
        if alloc.kind == "ExternalInput":
            if name != partition_name:
                in_names.append(name)
        elif alloc.kind == "ExternalOutput":
            out_names.append(name)
            shape = tuple(alloc.tensor_shape)
            dtype = mybir.dt.np(alloc.dtype)
            out_avals.append(jax.core.ShapedArray(shape, dtype))
            out_shapes.append((shape, dtype))
    n_params = len(in_names)
    n_outs = len(out_avals)
    in_names.extend(out_names)
    if partition_name is not None:
        in_names.append(partition_name)
    donate = tuple(range(n_params, n_params + n_outs))

    def _body(*a):
        operands = list(a)
        if partition_name is not None:
            operands.append(b2j.partition_id_tensor())
        outs = b2j._bass_exec_p.bind(
            *operands, out_avals=tuple(out_avals), in_names=tuple(in_names),
            out_names=tuple(out_names), lowering_input_output_aliases=(),
            sim_require_finite=True, sim_require_nnan=True, nc=nc)
        return tuple(outs)

    devices = jax.devices()[:NC]
    mesh = Mesh(np.asarray(devices), ("core",))
    sh = NamedSharding(mesh, PartitionSpec("core"))
    sharded = jax.jit(
        shard_map(_body, mesh=mesh,
                  in_specs=(PartitionSpec("core"),) * (n_params + n_outs),
                  out_specs=(PartitionSpec("core"),) * n_outs,
                  check_rep=False),
        donate_argnums=donate, keep_unused=True)
    concat_in = [np.concatenate([np.asarray(m[name]) for m in in_maps], axis=0)
                 for name in in_names[:n_params]]
    zeros = [np.zeros((NC * s[0], *s[1:]), d) for s, d in out_shapes]
    compiled = sharded.lower(*concat_in, *zeros).compile()

    dev_in = [jax.device_put(a, sh) for a in concat_in]
    dz_warm = [jax.device_put(z, sh) for z in zeros]
    dz_sets = [[jax.device_put(z, sh) for z in zeros] for _ in range(n_iter)]
    jax.block_until_ready(dev_in)
    jax.block_until_ready(dz_warm)
    jax.block_until_ready(dz_sets)

    warm = compiled(*dev_in, *dz_warm)
    jax.block_until_ready(warm)

    t0 = time.time()
    res = None
    for i in range(n_iter):
        res = compiled(*dev_in, *dz_sets[i])
    jax.block_until_ready(res)
    t1 = time.time()
    exec_ns = int((t1 - t0) / n_iter * 1e9)

    outs = []
    for c in range(NC):
        m = {}
        for i, name in enumerate(out_names):
            shape, _ = out_shapes[i]
            m[name] = np.asarray(warm[i]).reshape(NC, *shape)[c]
        outs.append(m["out"])
    return outs, exec_ns


def kernel(x, edge_index, W1, a_src1, a_dst1, b1, W2, a_src2, a_dst2, b2,
           W3, a_src3, a_dst3, b3):
    x = np.asarray(x, np.float32)
    edge_index = np.asarray(edge_index)
    args = [np.asarray(a, np.float32) for a in
            (W1, a_src1, a_dst1, W2, a_src2, a_dst2, W3, a_src3, a_dst3)]
    Wt, idx_w, xTo, Wc1, Wc2, Wc3, unperm = _prep(x, edge_index, *args)

    nc = bacc.Bacc(None, num_devices=NC, num_swdge_queues=4,
                   dynamic_dma_scratch_size=SCRATCH)
    nc = _build(nc, Wt)
    nc.compile()

    rowmask = (np.arange(NT)[None, :] * 128 +
               np.arange(128)[:, None] < NPC).astype(np.float32)
    b1p = np.asarray(b1, np.float32)[PERM36]
    b2p = np.asarray(b2, np.float32)[PERM36]
    in_maps = []
    for k in range(NC):
        in_maps.append({
            "idxw": idx_w[k],
            "xT_own": xTo[k],
            "wc1": Wc1, "wc2": Wc2, "wc3": Wc3,
            "b1b": np.broadcast_to(b1p, (128, HC)).copy(),
            "b2b": np.broadcast_to(b2p, (128, HC)).copy(),
            "b3b": np.broadcast_to(np.asarray(b3, np.float32), (128, D_IN)).copy(),
            "ident": np.eye(128, dtype=np.float16),
            "rowmask": rowmask,
        })
    global LAST_EXEC_NS
    try:
        outs, LAST_EXEC_NS = _run_timed(nc, in_maps)
    except Exception:
        import time as _time
        _t0 = _time.time()
        res = run_bass_kernel_spmd(nc, in_maps, core_ids=list(range(NC)))
        LAST_EXEC_NS = res.exec_time_ns or int((_time.time() - _t0) * 1e9)
        outs = [res.results[k]["out"] for k in range(NC)]
    stacked = np.concatenate([o[:NPC] for o in outs], axis=0)
    full = np.zeros((N, D_IN), np.float32)
    full[unperm] = stacked
    return full


# revision 20
# speedup vs baseline: 870.4019x; 1.1009x over previous
"""GAT 3-layer (DiffusionOrderingNetwork) Trainium2 kernel, 8-core SPMD.

Design (v2):
- Nodes partitioned 8x2500 by dst; per-core ELL (degree-sorted, per-tile
  width) edge layout; per-edge gathers via gpsimd dma_gather from an fp16
  HBM node table. Layers 1/2: 256B rows = [xw(36, (c,h)-major) | al_src(6)
  | pad]; layer 3 folds W3 into the table: 512B rows = [xw3/6 (192,
  (c,h)-major) | al_src3(6) | pad].
- Each core builds only its own 2560-row block (PE matmul vs fused
  [W | W@bd(a_src) | W@bd(a_dst)]), AllGathers the compact fp16 rows,
  then locally expands into the 256B/512B-strided gather table.
- Segment softmax uses a constant-shift exp (exp(lg-20), f32) -- no
  segment max needed; alpha = ex * (1/den) cast to fp16 before the
  payload multiply so the big DVE ops run in the 2x fp16 mode.
- Padding handled by two special table rows per core block: Z0 (zeros)
  and ZNEG (al_src = -1e4): pad slots gather ZNEG (ex -> 0), pad dst
  rows gather Z0 once so den > 0. No mask tensor at all.
- Layer 3: alpha-weighted sum of xw3 directly (outer product over
  (c=32, h=6)), head-mean = packed reduce; no per-head PE loop.
- Final global softmax over nodes via ones-matmul partition reduction +
  AllReduce + ones-matmul broadcast.
"""

import sys

sys.path.insert(0, "/opt/trn_rl_repo")

import numpy as np
import concourse.bass as bass
import concourse.bacc as bacc
import concourse.mybir as mybir
import concourse.tile as tile
import concourse.bass_isa as bass_isa
from concourse import library_config
from concourse.bass_utils import run_bass_kernel_spmd

N = 20000
NC = 8
NPC = 2500          # nodes per core
NT = 20             # node tiles per core (128 rows each; 2560 padded rows)
NPP = NT * 128      # 2560 padded rows per core
NTBL = NC * NPP     # 20480 table rows
H = 6
D_IN = 32
HC = 36             # heads * hidden
F32 = mybir.dt.float32
F16 = mybir.dt.float16
I16 = mybir.dt.int16
AF = mybir.ActivationFunctionType

CHUNK = 8           # gather chunk in ELL slots (128*CHUNK tokens per call)
SCRATCH = 32768     # SWDGE descriptor ring carveout (descs = SCRATCH//16)

import os as _os
ABLATE_COLL = _os.environ.get("ABLATE_COLL") == "1"   # skip collectives (debug)
ABLATE_GATH = _os.environ.get("ABLATE_GATH") == "1"   # skip gathers (debug)
ABLATE_BARR = _os.environ.get("ABLATE_BARR") == "1"   # skip barriers (debug)

# (c,h)-major permutations: new col j' = c*H + h holds old col h*C + c
PERM36 = np.array([(j % H) * 6 + j // H for j in range(36)])     # C=6
PERM192 = np.array([(j % H) * 32 + j // H for j in range(192)])  # C=32


def _blockdiag(a):
    # a: [H, C] -> [H*C, H] with col h = a[h] at rows h*C:(h+1)*C
    Hh, C = a.shape
    out = np.zeros((Hh * C, Hh), np.float32)
    for h in range(Hh):
        out[h * C:(h + 1) * C, h] = a[h]
    return out


def _prep(x, edge_index, W1, a_src1, a_dst1, W2, a_src2, a_dst2, W3, a_src3, a_dst3):
    src = np.concatenate([edge_index[0], np.arange(N)]).astype(np.int64)
    dst = np.concatenate([edge_index[1], np.arange(N)]).astype(np.int64)

    deg = np.bincount(dst, minlength=N)
    orders = []          # per core: local pos -> global node id
    global_pos = np.zeros(N, np.int64)
    for k in range(NC):
        d = deg[k * NPC:(k + 1) * NPC]
        order = np.argsort(-d, kind="stable") + k * NPC
        orders.append(order)
        global_pos[order] = k * NPP + np.arange(NPC)

    # shared tile width schedule (max across cores)
    Wt = np.zeros(NT, np.int64)
    for k in range(NC):
        ds_ = np.sort(deg[k * NPC:(k + 1) * NPC])[::-1]
        ds_ = np.concatenate([ds_, np.zeros(NPP - NPC, np.int64)])
        Wt = np.maximum(Wt, ds_.reshape(NT, 128).max(axis=1))
    Wt = np.maximum((Wt + 3) // 4 * 4, 4).astype(np.int64)

    # CSR by dst
    sort_by_dst = np.argsort(dst, kind="stable")
    src_s = src[sort_by_dst]
    rowptr = np.zeros(N + 1, np.int64)
    np.cumsum(deg, out=rowptr[1:])

    idx_w = []   # per core: [16, 8*sum(Wt)] int16 wrapped index stream
    for k in range(NC):
        order = orders[k]
        z0 = k * NPP + NPC        # all-zero row
        zneg = k * NPP + NPC + 1  # al_src = -1e4 row
        iw_parts = []
        for t in range(NT):
            w = int(Wt[t])
            ell = np.full((128, w), zneg, np.int64)
            for p in range(128):
                li = t * 128 + p
                if li < NPC:
                    n = order[li]
                    e0, e1 = rowptr[n], rowptr[n + 1]
                    dd = int(e1 - e0)
                    ell[p, :dd] = global_pos[src_s[e0:e1]]
                else:
                    ell[p, 0] = z0  # pad row: one live slot so den > 0
            stream = ell.T.reshape(-1)            # slot-major: s*128+p
            iw_parts.append(stream.reshape(-1, 16).T)   # [16, 8w]
        idx_w.append(np.concatenate(iw_parts, axis=1).astype(np.int16))

    # x in permuted order, padded, transposed: [32, NPP] f16 per core
    xp = np.zeros((NTBL, D_IN), np.float32)
    for k in range(NC):
        xp[k * NPP:k * NPP + NPC] = x[orders[k]]
    xTo = [np.ascontiguousarray(xp[k * NPP:(k + 1) * NPP].T).astype(np.float16)
           for k in range(NC)]

    Wc1 = np.concatenate(
        [W1[:, PERM36], W1 @ _blockdiag(a_src1), W1 @ _blockdiag(a_dst1)], 1)
    W2r = W2[PERM36, :]
    Wc2 = np.concatenate(
        [W2r[:, PERM36], W2r @ _blockdiag(a_src2), W2r @ _blockdiag(a_dst2)], 1)
    W3r = W3[PERM36, :]
    Wc3 = np.concatenate(
        [W3r[:, PERM192] / 6.0, W3r @ _blockdiag(a_src3), W3r @ _blockdiag(a_dst3)], 1)

    unperm = np.concatenate(orders)  # row i of stacked core outputs -> node id
    return Wt, idx_w, xTo, Wc1.astype(np.float16), Wc2.astype(np.float16), \
        Wc3.astype(np.float16), unperm


def _build(nc, Wt):
    SWt = int(Wt.sum())
    CIDX = 8 * SWt

    t_idx = nc.dram_tensor("idxw", [16, CIDX], I16, kind="ExternalInput")
    t_xTo = nc.dram_tensor("xT_own", [D_IN, NPP], F16, kind="ExternalInput")
    t_wc1 = nc.dram_tensor("wc1", [D_IN, 48], F16, kind="ExternalInput")
    t_wc2 = nc.dram_tensor("wc2", [HC, 48], F16, kind="ExternalInput")
    t_wc3 = nc.dram_tensor("wc3", [HC, 204], F16, kind="ExternalInput")
    t_b1 = nc.dram_tensor("b1b", [128, HC], F32, kind="ExternalInput")
    t_b2 = nc.dram_tensor("b2b", [128, HC], F32, kind="ExternalInput")
    t_b3 = nc.dram_tensor("b3b", [128, D_IN], F32, kind="ExternalInput")
    t_id = nc.dram_tensor("ident", [128, 128], F16, kind="ExternalInput")
    t_rm = nc.dram_tensor("rowmask", [128, NT], F32, kind="ExternalInput")
    t_out = nc.dram_tensor("out", [NPP, D_IN], F32, kind="ExternalOutput")

    with tile.TileContext(nc) as tc:
        with (
            tc.tile_pool(name="dram", bufs=1, space="DRAM") as dram,
            tc.tile_pool(name="cst", bufs=1) as cst,
            tc.tile_pool(name="gat", bufs=2) as gat,
            tc.tile_pool(name="stg", bufs=2) as stg,
            tc.tile_pool(name="wrkb", bufs=2) as wrkb,
            tc.tile_pool(name="wrk", bufs=3) as wrk,
            tc.tile_pool(name="acc", bufs=1) as acc,
            tc.tile_pool(name="ps", bufs=2, space="PSUM") as ps,
        ):
            nc.gpsimd.load_library(library_config.mlp)
            TBL12 = dram.tile([NTBL, 128], F16)
            TBL3 = dram.tile([NTBL, 256], F16, name="tbl3", tag="tbl3")
            BNC12 = dram.tile([NPP, 42], F16, name="bnc12", tag="bnc12")
            BNC3 = dram.tile([NPP, 198], F16, name="bnc3", tag="bnc3")
            TBLS12 = [dram.tile([NTBL, 42], F16, addr_space="Shared",
                                name="tbls1", tag="tbls1"),
                      dram.tile([NTBL, 42], F16, addr_space="Shared",
                                name="tbls2", tag="tbls2")]
            TBLS3 = dram.tile([NTBL, 198], F16, addr_space="Shared",
                              name="tbls3", tag="tbls3")
            CCI = dram.tile([1, 32], F32, name="cci", tag="cci")
            CCO = dram.tile([1, 32], F32, name="cco", tag="cco")

            # ---- constants ----
            sb_idx = cst.tile([128, CIDX], I16)
            for r in range(8):
                nc.sync.dma_start(out=sb_idx[16 * r:16 * (r + 1), :], in_=t_idx[:])
            sb_xTo = cst.tile([D_IN, NPP], F16)
            nc.sync.dma_start(out=sb_xTo[:], in_=t_xTo[:])
            sb_wc = [cst.tile([D_IN, 48], F16, tag="wc0", name="wc0"),
                     cst.tile([HC, 48], F16, tag="wc1t", name="wc1t"),
                     cst.tile([HC, 204], F16, tag="wc2t", name="wc2t")]
            nc.sync.dma_start(out=sb_wc[0][:], in_=t_wc1[:])
            nc.sync.dma_start(out=sb_wc[1][:], in_=t_wc2[:])
            nc.sync.dma_start(out=sb_wc[2][:], in_=t_wc3[:])
            sb_b = [cst.tile([128, HC], F32, tag="b0", name="b0"),
                    cst.tile([128, HC], F32, tag="b1t", name="b1t"),
                    cst.tile([128, D_IN], F32, tag="b2t", name="b2t")]
            nc.sync.dma_start(out=sb_b[0][:], in_=t_b1[:])
            nc.sync.dma_start(out=sb_b[1][:], in_=t_b2[:])
            nc.sync.dma_start(out=sb_b[2][:], in_=t_b3[:])
            ident = cst.tile([128, 128], F16)
            nc.sync.dma_start(out=ident[:], in_=t_id[:])
            sb_rm = cst.tile([128, NT], F32)
            nc.sync.dma_start(out=sb_rm[:], in_=t_rm[:])
            ones_r = cst.tile([1, 128], F32, tag="ones_r", name="ones_r")
            nc.vector.memset(ones_r[:], 1.0)
            ones_c = cst.tile([128, 1], F32, tag="ones_c", name="ones_c")
            nc.vector.memset(ones_c[:], 1.0)
            zrow = cst.tile([1, 256], F16, tag="zrow", name="zrow")
            nc.vector.memset(zrow[:], 0.0)
            nrow12 = cst.tile([1, 42], F16, tag="nrow12", name="nrow12")
            nc.vector.memset(nrow12[:], 0.0)
            nc.vector.memset(nrow12[:, 36:42], -10000.0)
            nrow3 = cst.tile([1, 198], F16, tag="nrow3", name="nrow3")
            nc.vector.memset(nrow3[:], 0.0)
            nc.vector.memset(nrow3[:, 192:198], -10000.0)
            bm20 = cst.tile([128, 1], F32, tag="bm20", name="bm20")
            nc.vector.memset(bm20[:], -20.0)
            bm50 = cst.tile([128, 1], F32, tag="bm50", name="bm50")
            nc.vector.memset(bm50[:], -50.0)

            # persistent per-layer state
            ald_own = [acc.tile([128, NT, H], F16, tag="ald0", name="ald0"),
                       acc.tile([128, NT, H], F16, tag="ald1", name="ald1")]
            h_all = acc.tile([128, NT, HC], F16)
            e3_all = acc.tile([128, NT, D_IN], F32)

            def build_tile(li, t, lhsT):
                # matmul vs fused Wc -> table row block + own al_dst
                wcols = 204 if li == 2 else 48
                pw = 198 if li == 2 else 42
                pt = ps.tile([128, 204], F32, tag="tb")
                nc.tensor.matmul(pt[:, 0:wcols], lhsT, sb_wc[li][:],
                                 start=True, stop=True)
                nc.scalar.activation(ald_own[li % 2][:, t, :],
                                     pt[:, pw:pw + H], AF.Copy)
                tb = wrk.tile([128, 198], F16, tag="tbs")
                nc.scalar.activation(tb[:, 0:pw], pt[:, 0:pw], AF.Copy)
                bnc = BNC3 if li == 2 else BNC12
                nc.sync.dma_start(out=bnc[t * 128:(t + 1) * 128, 0:pw],
                                  in_=tb[:, 0:pw])

            # ---- initial (layer-1) table from own x rows ----
            for t in range(NT):
                build_tile(0, t, sb_xTo[:, t * 128:(t + 1) * 128])
            nc.sync.dma_start(out=BNC12[NPC:NPC + 1, :], in_=zrow[:, 0:42])
            nc.sync.dma_start(out=BNC12[NPC + 1:NPC + 2, :], in_=nrow12[:])

            def allgather_expand(li):
                tbls = TBLS3 if li == 2 else TBLS12[li]
                bnc = BNC3 if li == 2 else BNC12
                tbl = TBL3 if li == 2 else TBL12
                pw = 198 if li == 2 else 42
                tw = 256 if li == 2 else 128
                if not ABLATE_BARR:
                    tc.strict_bb_all_engine_barrier()
                if not ABLATE_COLL:
                    nc.gpsimd.collective_compute(
                        "AllGather", mybir.AluOpType.bypass,
                        replica_groups=[list(range(NC))],
                        ins=[bnc[:].opt()], outs=[tbls[:].opt()])
                if not ABLATE_BARR:
                    tc.strict_bb_all_engine_barrier()
                # expand compact rows into the 256B/512B-strided gather table
                nblk = NTBL // 128          # 160 row-blocks of 128
                step = 40                   # blocks per staging chunk
                for b0 in range(0, nblk, step):
                    st = stg.tile([128, 8192], F16, tag="stage")
                    sv = st[:, 0:step * pw].rearrange("p (b c) -> p b c", c=pw)
                    nc.sync.dma_start(
                        out=sv,
                        in_=tbls[b0 * 128:(b0 + step) * 128, :]
                        .rearrange("(b p) c -> p b c", p=128))
                    nc.sync.dma_start(
                        out=tbl[b0 * 128:(b0 + step) * 128, :]
                        .rearrange("(b p) c -> p b c", p=128)[:, :, 0:pw],
                        in_=sv)

            allgather_expand(0)

            # ---- layers ----
            qctr = 0
            for li in range(3):
                tbl = TBL3 if li == 2 else TBL12
                el = 256 if li == 2 else 128
                pw = 192 if li == 2 else 36
                ioff = 0
                for t in range(NT):
                    w = int(Wt[t])
                    GU = gat.tile([128, 15360], F16, tag="G")
                    G = GU.rearrange("p (s e) -> p s e", e=el)
                    for c in range(0, w, CHUNK):
                        cw = min(CHUNK, w - c)
                        cni = 128 * cw
                        if not (ABLATE_GATH and c > 0):
                            nc.gpsimd.dma_gather(
                                out_ap=G[:, c:c + cw, :],
                                in_ap=tbl[:],
                                idxs_ap=sb_idx[:, ioff + 8 * c:ioff + 8 * (c + cw)],
                                num_idxs=cni, num_idxs_reg=cni, elem_size=el,
                                queue_num=qctr % 4,
                            )
                        qctr += 1
                    # attention: lg = al_src[src] + al_dst[dst]
                    lg = wrk.tile([128, 60, H], F16, tag="lg")
                    nc.vector.tensor_tensor(
                        out=lg[:, 0:w, :], in0=G[:, 0:w, pw:pw + H],
                        in1=ald_own[li % 2][:, t, :][:, None, :]
                        .broadcast_to([128, w, H]),
                        op=mybir.AluOpType.add)
                    lr = wrk.tile([128, 60, H], F16, tag="lr")
                    nc.scalar.activation(lr[:, 0:w, :], lg[:, 0:w, :],
                                         AF.Lrelu, alpha=0.2)
                    ex = wrk.tile([128, 60, H], F32, tag="ex")
                    nc.scalar.activation(ex[:, 0:w, :], lr[:, 0:w, :],
                                         AF.Exp, bias=bm20[:])
                    den = wrk.tile([128, H], F32, tag="den")
                    nc.vector.tensor_reduce(
                        out=den[:], in_=ex[:, 0:w, :].rearrange("p s h -> p h s"),
                        axis=mybir.AxisListType.X, op=mybir.AluOpType.add)
                    rd = wrk.tile([128, H], F32, tag="rd")
                    nc.vector.reciprocal(rd[:], den[:])
                    al16 = wrk.tile([128, 60, H], F16, tag="al16")
                    nc.vector.tensor_tensor(
                        out=al16[:, 0:w, :], in0=ex[:, 0:w, :],
                        in1=rd[:][:, None, :].broadcast_to([128, w, H]),
                        op=mybir.AluOpType.mult)
                    # alpha-weighted payload aggregation ((c,h)-major)
                    cdim = pw // H
                    msg = wrkb.tile([128, 60, 192], F16, tag="msg")
                    nc.vector.tensor_tensor(
                        out=msg[:, 0:w, 0:pw]
                        .rearrange("p s (c h) -> p s c h", h=H),
                        in0=G[:, 0:w, 0:pw]
                        .rearrange("p s (c h) -> p s c h", h=H),
                        in1=al16[:, 0:w, :][:, :, None, :]
                        .broadcast_to([128, w, cdim, H]),
                        op=mybir.AluOpType.mult)
                    agg = wrk.tile([128, 192], F32, tag="agg")
                    nc.vector.tensor_reduce(
                        out=agg[:, 0:pw],
                        in_=msg[:, 0:w, 0:pw].rearrange("p s j -> p j s"),
                        axis=mybir.AxisListType.X, op=mybir.AluOpType.add)
                    if li < 2:
                        hp = wrk.tile([128, HC], F32, tag="hp")
                        nc.vector.tensor_tensor(out=hp[:], in0=agg[:, 0:HC],
                                                in1=sb_b[li][:],
                                                op=mybir.AluOpType.add)
                        nc.scalar.activation(h_all[:, t, :], hp[:], AF.Relu)
                        # build next layer's table rows for this tile
                        tp = ps.tile([HC, 128], F16, tag="tp")
                        nc.tensor.transpose(tp[:], h_all[:, t, :], ident[:])
                        ts16 = wrk.tile([HC, 128], F16, tag="ts16")
                        nc.scalar.activation(ts16[:], tp[:], AF.Copy)
                        build_tile(li + 1, t, ts16[:])
                    else:
                        zs = wrk.tile([128, D_IN], F32, tag="zs")
                        nc.vector.tensor_reduce(
                            out=zs[:],
                            in_=agg[:].rearrange("p (c h) -> p c h", h=H),
                            axis=mybir.AxisListType.X, op=mybir.AluOpType.add)
                        nc.vector.tensor_tensor(out=zs[:], in0=zs[:],
                                                in1=sb_b[2][:],
                                                op=mybir.AluOpType.add)
                        nc.scalar.activation(e3_all[:, t, :], zs[:],
                                             AF.Exp, bias=bm50[:])
                    ioff += 8 * w

                if li < 2:
                    bnc = BNC3 if li == 1 else BNC12
                    pwb = 198 if li == 1 else 42
                    nrow = nrow3 if li == 1 else nrow12
                    nc.sync.dma_start(out=bnc[NPC:NPC + 1, 0:pwb],
                                      in_=zrow[:, 0:pwb])
                    nc.sync.dma_start(out=bnc[NPC + 1:NPC + 2, 0:pwb],
                                      in_=nrow[:])
                    allgather_expand(li + 1)

            # ---- global softmax over nodes ----
            nc.vector.tensor_tensor(
                out=e3_all[:], in0=e3_all[:],
                in1=sb_rm[:][:, :, None].broadcast_to([128, NT, D_IN]),
                op=mybir.AluOpType.mult)
            s0 = wrk.tile([128, D_IN], F32, tag="s0")
            nc.vector.tensor_reduce(out=s0[:],
                                    in_=e3_all[:].rearrange("p t c -> p c t"),
                                    axis=mybir.AxisListType.X,
                                    op=mybir.AluOpType.add)
            sp = ps.tile([1, D_IN], F32, tag="sp")
            nc.tensor.matmul(sp[:], ones_c[:], s0[:], start=True, stop=True)
            red = wrk.tile([1, D_IN], F32, tag="red")
            nc.scalar.activation(red[:], sp[:], AF.Copy)
            nc.sync.dma_start(out=CCI[:], in_=red[:])
            if not ABLATE_BARR:
                tc.strict_bb_all_engine_barrier()
            if not ABLATE_COLL:
                nc.gpsimd.collective_compute(
                    "AllReduce", mybir.AluOpType.add,
                    replica_groups=[list(range(NC))],
                    ins=[CCI[:].opt()], outs=[CCO[:].opt()])
            else:
                nc.sync.dma_start(out=CCO[:], in_=red[:])
            if not ABLATE_BARR:
                tc.strict_bb_all_engine_barrier()
            ssum = wrk.tile([1, D_IN], F32, tag="ssum")
            nc.sync.dma_start(out=ssum[:], in_=CCO[:])
            rc = wrk.tile([1, D_IN], F32, tag="rc")
            nc.vector.reciprocal(rc[:], ssum[:])
            rbp = ps.tile([128, D_IN], F32, tag="rbp")
            nc.tensor.matmul(rbp[:], ones_r[:], rc[:], start=True, stop=True)
            rb = wrk.tile([128, D_IN], F32, tag="rb")
            nc.scalar.activation(rb[:], rbp[:], AF.Copy)
            ot = wrk.tile([128, NT, D_IN], F32, tag="ot")
            nc.vector.tensor_tensor(
                out=ot[:], in0=e3_all[:],
                in1=rb[:][:, None, :].broadcast_to([128, NT, D_IN]),
                op=mybir.AluOpType.mult)
            nc.sync.dma_start(
                out=t_out[:].rearrange("(t p) c -> p t c", p=128), in_=ot[:])
    return nc


_CACHE = {}
LAST_EXEC_NS = None
LAST_TRACE_DIR = None


def _run_timed(nc, in_maps, n_iter=32):
    """Execute the compiled SPMD kernel and measure HW execution time.

    No NTFF profiling hook exists under this axon tunnel, so
    neuron-profile exec_time_ns is unavailable. Closest honest proxy:
    pre-stage all inputs in device HBM (NTFF exec time excludes host
    transfers too), then time n_iter back-to-back executions of the
    compiled NEFF on all 8 cores and report the per-iteration mean.
    The one-time ~85 ms axon RPC latency is excluded via a warmup run;
    outputs are taken from the warmup execution.
    """
    import time
    import jax
    from jax.sharding import Mesh, PartitionSpec, NamedSharding
    from jax.experimental.shard_map import shard_map
    import concourse.bass2jax as b2j

    b2j.install_neuronx_cc_hook()
    partition_name = nc.partition_id_tensor.name if nc.partition_id_tensor else None
    in_names, out_names, out_avals, out_shapes = [], [], [], []
    for alloc in nc.m.functions[0].allocations:
        if not isinstance(alloc, mybir.MemoryLocationSet):
            continue
        name = alloc.memorylocations[0].name
        if alloc.kind == "ExternalInput":
            if name != partition_name:
                in_names.append(name)
        elif alloc.kind == "ExternalOutput":
            out_names.append(name)
            shape = tuple(alloc.tensor_shape)
            dtype = mybir.dt.np(alloc.dtype)
            out_avals.append(jax.core.ShapedArray(shape, dtype))
            out_shapes.append((shape, dtype))
    n_params = len(in_names)
    n_outs = len(out_avals)
    in_names.extend(out_names)
    if partition_name is not None:
        in_names.append(partition_name)
    donate = tuple(range(n_params, n_params + n_outs))

    def _body(*a):
        operands = list(a)
        if partition_name is not None:
            operands.append(b2j.partition_id_tensor())
        outs = b2j._bass_exec_p.bind(
            *operands, out_avals=tuple(out_avals), in_names=tuple(in_names),
            out_names=tuple(out_names), lowering_input_output_aliases=(),
            sim_require_finite=True, sim_require_nnan=True, nc=nc)
        return tuple(outs)

    devices = jax.devices()[:NC]
    mesh = Mesh(np.asarray(devices), ("core",))
    sh = NamedSharding(mesh, PartitionSpec("core"))
    sharded = jax.jit(
        shard_map(_body, mesh=mesh,
                  in_specs=(PartitionSpec("core"),) * (n_params + n_outs),
                  out_specs=(PartitionSpec("core"),) * n_outs,
                  check_rep=False),
        donate_argnums=donate, keep_unused=True)
    concat_in = [np.concatenate([np.asarray(m[name]) for m in in_maps], axis=0)
                 for name in in_names[:n_params]]
    zeros = [np.zeros((NC * s[0], *s[1:]), d) for s, d in out_shapes]
    compiled = sharded.lower(*concat_in, *zeros).compile()

    dev_in = [jax.device_put(a, sh) for a in concat_in]
    dz_warm = [jax.device_put(z, sh) for z in zeros]
    dz_sets = [[jax.device_put(z, sh) for z in zeros] for _ in range(n_iter)]
    jax.block_until_ready(dev_in)
    jax.block_until_ready(dz_warm)
    jax.block_until_ready(dz_sets)

    warm = compiled(*dev_in, *dz_warm)
    jax.block_until_ready(warm)

    t0 = time.time()
    res = None
    for i in range(n_iter):
        res = compiled(*dev_in, *dz_sets[i])
    jax.block_until_ready(res)
    t1 = time.time()
    exec_ns = int((t1 - t0) / n_iter * 1e9)

    outs = []
    for c in range(NC):
        m = {}
        for i, name in enumerate(out_names):
            shape, _ = out_shapes[i]
            m[name] = np.asarray(warm[i]).reshape(NC, *shape)[c]
        outs.append(m["out"])
    return outs, exec_ns


def kernel(x, edge_index, W1, a_src1, a_dst1, b1, W2, a_src2, a_dst2, b2,
           W3, a_src3, a_dst3, b3):
    x = np.asarray(x, np.float32)
    edge_index = np.asarray(edge_index)
    args = [np.asarray(a, np.float32) for a in
            (W1, a_src1, a_dst1, W2, a_src2, a_dst2, W3, a_src3, a_dst3)]
    Wt, idx_w, xTo, Wc1, Wc2, Wc3, unperm = _prep(x, edge_index, *args)

    nc = bacc.Bacc(None, num_devices=NC, num_swdge_queues=4,
                   dynamic_dma_scratch_size=SCRATCH)
    nc = _build(nc, Wt)
    nc.compile()

    rowmask = (np.arange(NT)[None, :] * 128 +
               np.arange(128)[:, None] < NPC).astype(np.float32)
    b1p = np.asarray(b1, np.float32)[PERM36]
    b2p = np.asarray(b2, np.float32)[PERM36]
    in_maps = []
    for k in range(NC):
        in_maps.append({
            "idxw": idx_w[k],
            "xT_own": xTo[k],
            "wc1": Wc1, "wc2": Wc2, "wc3": Wc3,
            "b1b": np.broadcast_to(b1p, (128, HC)).copy(),
            "b2b": np.broadcast_to(b2p, (128, HC)).copy(),
            "b3b": np.broadcast_to(np.asarray(b3, np.float32), (128, D_IN)).copy(),
            "ident": np.eye(128, dtype=np.float16),
            "rowmask": rowmask,
        })
    global LAST_EXEC_NS
    try:
        outs, LAST_EXEC_NS = _run_timed(nc, in_maps)
    except Exception:
        import time as _time
        _t0 = _time.time()
        res = run_bass_kernel_spmd(nc, in_maps, core_ids=list(range(NC)))
        LAST_EXEC_NS = res.exec_time_ns or int((_time.time() - _t0) * 1e9)
        outs = [res.results[k]["out"] for k in range(NC)]
    stacked = np.concatenate([o[:NPC] for o in outs], axis=0)
    full = np.zeros((N, D_IN), np.float32)
    full[unperm] = stacked
    return full
